# revision 26
# baseline (speedup 1.0000x reference)
"""Trainium2 Bass kernel for nn_LlamaAttention_kvcache (sparse H2O attention).

Strategy (8 NeuronCores, tensor-parallel over heads, 4 heads/core):

Phase 1 (device, fp32 storage / fp32r matmuls -- 1 cyc/row at free>=256,
same PE speed as bf16 but TF32-grade mantissa so the per-head top-k
selection matches the fp32 reference exactly):
  q/k projections (scale folded into Wq), RoPE, causal-skipped QK^T
  (only k-chunks at/below the diagonal; diagonal chunk gets an additive
  -1e9 triangular mask), exp with row-accumulation, per-head column
  scores  scores[j] = sum_i exp(aw[i,j]) / r_i  via r^T @ E matmuls.

Host: exact top-k per head (jax.lax.top_k tie semantics), gathers kept
x-rows, builds the prefix matrix map.

Phase 2 (device, bf16): the reference output is
    out = sum_kept (aw+1e9) v  -  1e9 * sum_all v      (per row, exactly)
The (aw+1e9) factor is 1e9 * causal-step + O(10) where the O(10) part
is ~1e-9 of the output scale (expected absmax ~1.8e11), far below fp32
resolution of the accumulated sum, so the device computes the step part:
    po[:, i] = 1e9 * sum_{kept j <= i} v_j
which is piecewise-constant in i with <= NKEPT+1 distinct values.  So
phase 2 computes v for kept tokens, prefix sums via a triangular ones
matmul, and only the <=256 distinct o_proj rows per head:
    outS_h = (1e9 * cumsum(v_h)) @ Wo_h          [256, 4096]
Host expands rows (gather), sums the 8 cores' partials, and adds the
exact -1e9 * (sum_all v) @ Wo rank-1 bias row computed in fp64.
"""

import contextlib
import os
import sys

for p in ("/opt/trn_rl_repo", "/root/.axon_site/_ro/trn_rl_repo"):
    if p not in sys.path:
        sys.path.append(p)

import numpy as np
import ml_dtypes

import concourse.bacc as bacc
import concourse.mybir as mybir
import concourse.tile as tile
from concourse.bass_utils import run_bass_kernel_spmd

F32 = mybir.dt.float32
F32R = mybir.dt.float32r
BF16 = mybir.dt.bfloat16
BF = ml_dtypes.bfloat16

P = 128
S = 2048
H = 4096
NH = 32
HD = 128
NCORES = 8
HPC = NH // NCORES          # heads per core = 4
KC = H // P                 # 32 contraction chunks
KEEP = int(0.1 * S)         # 204 top-k heavy hitters
NKEPT = KEEP + 2            # + last-2 local tokens = 206
KPAD = 256                  # padded kept count

_cache = {}


def _r(ap):
    return ap   # operands already declared float32r


def _build_phase1():
    nc = bacc.Bacc("TRN2", target_bir_lowering=False, debug=False,
                   num_devices=NCORES)
    xt = nc.dram_tensor("xt", [H, S], F32R, kind="ExternalInput").ap()
    wqk = nc.dram_tensor("wqk", [H, 2 * HPC * HD], F32R,
                         kind="ExternalInput").ap()
    cosd = nc.dram_tensor("cos", [P, S], F32, kind="ExternalInput").ap()
    sind = nc.dram_tensor("sin", [P, S], F32, kind="ExternalInput").ap()
    dmaskd = nc.dram_tensor("dmask", [4 * P, 512], F32,
                            kind="ExternalInput").ap()
    scores_o = nc.dram_tensor("scores", [HPC, S], F32,
                              kind="ExternalOutput").ap()

    with tile.TileContext(nc) as tc, contextlib.ExitStack() as ctx, \
         nc.allow_low_precision(reason="hand-analyzed tf32 score path"):
        # DRAM round-trip buffer for roped q/k (dep-tracked tile):
        # rows [0:512] = qT (4 heads x 128 d), rows [512:1024] = kT.
        dpool = ctx.enter_context(tc.tile_pool(name="dpool", bufs=1,
                                               space="DRAM"))
        qkd = dpool.tile([2 * HPC * HD, S], F32R, name="qkd", tag="qkd")

        # ---- stage A: projections + rope (PE: 32kc x 8 x 4 s-chunks)
        with tc.tile_pool(name="wpool", bufs=1) as wpool, \
             tc.tile_pool(name="cpool", bufs=1) as cpool, \
             tc.tile_pool(name="xpool", bufs=4) as xpool, \
             tc.tile_pool(name="rpool", bufs=2) as rpool, \
             tc.tile_pool(name="stpool", bufs=4) as stpool, \
             tc.tile_pool(name="ppool", bufs=1, space="PSUM") as ppool:
            # weights on the scalar+gpsimd DGE queues so x chunks
            # (sync queue) are not stuck behind 16.8 MB of weight traffic
            w_sb = []
            wq_engines = (nc.scalar, nc.gpsimd)
            for kc in range(KC):
                t = wpool.tile([P, 2 * HPC * HD], F32R, name=f"w{kc}",
                               tag=f"w{kc}")
                wq_engines[kc % 2].dma_start(t[:], wqk[kc * P:(kc + 1) * P, :])
                w_sb.append(t)
            # cos/sin behind the gpsimd weight stream; needed only at ~50us
            cos_sb = cpool.tile([P, S], F32, name="cos_sb", tag="cos")
            sin_sb = cpool.tile([P, S], F32, name="sin_sb", tag="sin")
            nc.gpsimd.dma_start(cos_sb[:], cosd[:, :])
            nc.gpsimd.dma_start(sin_sb[:], sind[:, :])
            IORD = (4, 0, 5, 1, 6, 2, 7, 3)   # k-heads first
            for sq in range(4):
                ssl = slice(sq * 512, (sq + 1) * 512)
                ps = [ppool.tile([P, 512], F32, name=f"pj{i}", tag=f"pj{i}")
                      for i in range(8)]
                for kc in range(KC):
                    xc = xpool.tile([P, 512], F32R, name="xc", tag="xc")
                    nc.sync.dma_start(xc[:], xt[kc * P:(kc + 1) * P, ssl])
                    for i in IORD:
                        nc.tensor.matmul(
                            ps[i][:], lhsT=_r(w_sb[kc][:, i * HD:(i + 1) * HD]),
                            rhs=_r(xc[:]), start=(kc == 0), stop=(kc == KC - 1))
                # first release all 8 PSUM banks with plain copies
                # (alternating DVE/ACT), then do the rope math on the copies
                cps = {}
                for n, i in enumerate(IORD):
                    cp = rpool.tile([P, 512], F32, name="ropecp",
                                    tag=f"ropecp{i}", bufs=2)
                    if n % 2 == 0:
                        nc.vector.tensor_copy(cp[:], ps[i][:])
                    else:
                        nc.scalar.activation(cp[:], ps[i][:],
                                             mybir.ActivationFunctionType.Copy)
                    cps[i] = cp
                for i in IORD:
                    cp = cps[i]
                    m = rpool.tile([P, 512], F32, name="ropem", tag="ropem")
                    nc.vector.tensor_mul(m[:], cp[:], cos_sb[:, ssl])
                    rot = rpool.tile([P, 512], F32, name="roper", tag="roper")
                    # rotate-half on the otherwise-idle scalar engine
                    nc.scalar.activation(rot[0:64, :], cp[64:128, :],
                                         mybir.ActivationFunctionType.Copy,
                                         scale=-1.0)
                    nc.scalar.activation(rot[64:128, :], cp[0:64, :],
                                         mybir.ActivationFunctionType.Copy,
                                         scale=1.0)
                    rs_ = rpool.tile([P, 512], F32, name="ropes", tag="ropes")
                    nc.vector.tensor_mul(rs_[:], rot[:], sin_sb[:, ssl])
                    st = stpool.tile([P, 512], F32R, name="strope", tag="strope")
                    nc.vector.tensor_add(st[:], m[:], rs_[:])
                    nc.sync.dma_start(qkd[i * HD:(i + 1) * HD, ssl], st[:])

        # ---- stage B: causal QK^T + exp + column scores.
        # Two-pass, software-pipelined quarter-head units: pass 1 computes
        # QK chunks + exp (E retained in SBUF) + row sums; pass 2 (one unit
        # behind) does the r^T E score matmuls.  PSUM: aw 2x[128,1024] +
        # one [1,S] score row = 8 banks.
        with tc.tile_pool(name="ktp", bufs=2) as ktp, \
             tc.tile_pool(name="qbp", bufs=2) as qbp, \
             tc.tile_pool(name="epool", bufs=int(os.environ.get("KEBUF", "3"))) as epool, \
             tc.tile_pool(name="vp", bufs=4) as vp, \
             tc.tile_pool(name="rvp", bufs=16) as rvp, \
             tc.tile_pool(name="dmp", bufs=1) as dmp, \
             tc.tile_pool(name="scp", bufs=2) as scp, \
             tc.tile_pool(name="spool", bufs=1, space="PSUM") as spool, \
             tc.tile_pool(name="apool", bufs=2, space="PSUM") as apool:
            dm_sb = [dmp.tile([P, 512], F32, name=f"dm{j}", tag=f"dm{j}")
                     for j in range(4)]
            for j in range(4):
                nc.sync.dma_start(dm_sb[j][:], dmaskd[j * P:(j + 1) * P, :])

            UQT = int(os.environ.get("KUQT", "4"))
            ESZ = {2: 3072, 4: 8192, 8: 14336, 16: 20480}[UQT]
            state = {}

            def emit_p1(h, q):
                if q == 0:
                    kt = ktp.tile([P, S], F32R, name="kt", tag="kt")
                    qbt = qbp.tile([P, S], F32R, name="qbt", tag="qbt")
                    # column pieces in sq order on the gpsimd queue: piece sq
                    # is ready as soon as stage A's s-chunk sq is written, and
                    # the queue has no write backlog, so early pieces land
                    # immediately (only piece 3 waits on the rope tail)
                    for sq in range(4):
                        kssl = slice(sq * 512, (sq + 1) * 512)
                        nc.gpsimd.dma_start(
                            kt[:, kssl],
                            qkd[(HPC + h) * HD:(HPC + h + 1) * HD, kssl])
                        nc.gpsimd.dma_start(
                            qbt[:, kssl],
                            qkd[h * HD:(h + 1) * HD, kssl])
                    state[h] = {"kt": kt, "qbt": qbt, "rinv": {}, "E": {}}
                st_ = state[h]
                E = epool.tile([P, ESZ], F32R, name="Eu", tag="Eu")
                st_["E"][q] = E
                eoff = 0
                for qt in range(UQT * q, UQT * q + UQT):
                    W = (qt // 4 + 1) * 512
                    rs_list = []
                    for off in range(0, W, 1024):
                        w = min(1024, W - off)
                        aw = apool.tile([P, 1024], F32, name="aw", tag="aw")
                        for sub in range(0, w, 512):
                            nc.tensor.matmul(
                                aw[:, sub:sub + 512],
                                lhsT=_r(st_["qbt"][:, qt * P:(qt + 1) * P]),
                                rhs=_r(st_["kt"][:, off + sub:off + sub + 512]),
                                start=True, stop=True)
                        if off + w == W:   # diagonal in the final 512 cols
                            nc.vector.tensor_add(aw[:, w - 512:w],
                                                 aw[:, w - 512:w],
                                                 dm_sb[qt % 4][:])
                        rs_ = vp.tile([P, 1], F32, name="rsp",
                                      tag=f"rsp{off // 1024}")
                        nc.scalar.activation(E[:, eoff + off:eoff + off + w],
                                             aw[:, :w],
                                             mybir.ActivationFunctionType.Exp,
                                             accum_out=rs_[:])
                        rs_list.append(rs_)
                    rtot = rs_list[0]
                    for c in range(1, len(rs_list)):
                        nr = vp.tile([P, 1], F32, name="racc", tag=f"racc{c}")
                        nc.vector.tensor_add(nr[:], rtot[:], rs_list[c][:])
                        rtot = nr
                    rinv = rvp.tile([P, 1], F32R, name="rinv", tag="rinv")
                    nc.vector.reciprocal(rinv[:], rtot[:])
                    st_["rinv"][qt] = rinv
                    eoff += W

            def emit_p2(h, q):
                st_ = state[h]
                if q == 0:
                    st_["sc"] = spool.tile([1, S], F32, name="scps",
                                           tag="scps")
                E = st_["E"][q]
                eoff = 0
                for qt in range(UQT * q, UQT * q + UQT):
                    W = (qt // 4 + 1) * 512
                    rinv = st_["rinv"][qt]
                    for c in range(W // 512):
                        nc.tensor.matmul(
                            st_["sc"][:, c * 512:(c + 1) * 512],
                            lhsT=_r(rinv[:]),
                            rhs=_r(E[:, eoff + c * 512:eoff + (c + 1) * 512]),
                            start=(qt == 4 * c), stop=(qt == 15))
                    eoff += W
                if q == 16 // UQT - 1:
                    scsb = scp.tile([1, S], F32, name="scsb", tag="scsb")
                    nc.vector.tensor_copy(scsb[:], st_["sc"][:])
                    nc.gpsimd.dma_start(scores_o[h:h + 1, :], scsb[:])

            units = [(h, q) for h in range(HPC) for q in range(16 // UQT)]
            for idx, (h, q) in enumerate(units):
                emit_p1(h, q)
                if idx >= 1:
                    emit_p2(*units[idx - 1])
            emit_p2(*units[-1])
    nc.compile()
    return nc


def _build_phase2():
    nc = bacc.Bacc("TRN2", target_bir_lowering=False, debug=False,
                   num_devices=NCORES)
    xtk = nc.dram_tensor("xtk", [H, HPC * KPAD], BF16,
                         kind="ExternalInput").ap()
    wv = nc.dram_tensor("wv", [H, HPC * HD], BF16, kind="ExternalInput").ap()
    wo = nc.dram_tensor("wo", [HPC * HD, H], BF16, kind="ExternalInput").ap()
    Rmd = nc.dram_tensor("Rm", [KPAD, KPAD], BF16, kind="ExternalInput").ap()
    outS = nc.dram_tensor("outS", [HPC * KPAD, H], BF16,
                          kind="ExternalOutput").ap()

    with tile.TileContext(nc) as tc, contextlib.ExitStack() as ctx:
        wvp = ctx.enter_context(tc.tile_pool(name="wvp", bufs=1))
        wop = ctx.enter_context(tc.tile_pool(name="wop", bufs=1))
        xkp = ctx.enter_context(tc.tile_pool(name="xkp", bufs=6))
        rp = ctx.enter_context(tc.tile_pool(name="rp", bufs=1))
        vsb = ctx.enter_context(tc.tile_pool(name="vsb", bufs=1))
        csb = ctx.enter_context(tc.tile_pool(name="csb", bufs=1))
        osb = ctx.enter_context(tc.tile_pool(name="osb", bufs=6))

        R_sb = [rp.tile([P, KPAD], BF16, name=f"R{t}", tag=f"R{t}")
                for t in range(2)]
        for t in range(2):
            nc.scalar.dma_start(R_sb[t][:], Rmd[t * P:(t + 1) * P, :])
        # wv on the scalar queue so xtk chunks (sync queue) start immediately
        wv_sb = []
        for kc in range(KC):
            t = wvp.tile([P, HPC * HD], BF16, name=f"wv{kc}", tag=f"wv{kc}")
            nc.scalar.dma_start(t[:], wv[kc * P:(kc + 1) * P, :])
            wv_sb.append(t)

        # v projection of kept tokens: v_sb[h][t] = [128 kept, 128 d] bf16
        v_sb = [[vsb.tile([P, HD], BF16, name=f"vsb{h}_{t}", tag=f"vsb{h}_{t}")
                 for t in range(2)] for h in range(HPC)]
        with tc.tile_pool(name="vps", bufs=1, space="PSUM") as vps:
            v_ps = [[vps.tile([P, HD], F32, name=f"vps{h}_{t}",
                              tag=f"vps{h}_{t}")
                     for t in range(2)] for h in range(HPC)]
            for kc in range(KC):
                xk = xkp.tile([P, HPC * KPAD], BF16, name="xk", tag="xk")
                nc.sync.dma_start(xk[:], xtk[kc * P:(kc + 1) * P, :])
                for h in range(HPC):
                    for t in range(2):
                        nc.tensor.matmul(
                            v_ps[h][t][:],
                            lhsT=xk[:, h * KPAD + t * P:h * KPAD + (t + 1) * P],
                            rhs=wv_sb[kc][:, h * HD:(h + 1) * HD],
                            start=(kc == 0), stop=(kc == KC - 1))
            for h in range(HPC):
                for t in range(2):
                    nc.vector.tensor_copy(v_sb[h][t][:], v_ps[h][t][:])

        # wo loads emitted after the v-proj stream so they don't delay it
        wo_sb = [wop.tile([P, H], BF16, name=f"wo{h}", tag=f"wo{h}")
                 for h in range(HPC)]
        for h in range(HPC):
            nc.gpsimd.dma_start(wo_sb[h][:], wo[h * P:(h + 1) * P, :])

        # prefix sums over sorted kept order: cumT[h] = [128 d, 256 m] bf16,
        # scaled by 1e9 on the PSUM->SBUF copy.
        cum_sb = [csb.tile([P, KPAD], BF16, name=f"cum{h}", tag=f"cum{h}")
                  for h in range(HPC)]
        with tc.tile_pool(name="cps", bufs=1, space="PSUM") as cps:
            for h in range(HPC):
                cum_ps = cps.tile([P, KPAD], F32, name="cumps", tag=f"cps{h}")
                for t in range(2):
                    nc.tensor.matmul(cum_ps[:], lhsT=v_sb[h][t][:],
                                     rhs=R_sb[t][:],
                                     start=(t == 0), stop=(t == 1))
                nc.scalar.activation(cum_sb[h][:], cum_ps[:],
                                     mybir.ActivationFunctionType.Copy,
                                     scale=1e9)

        # distinct o_proj rows: outS[h*256+m, :] = cumT[h][:, m] @ wo_h
        # only m <= NKEPT=206 is ever gathered, so the mb=1 block writes
        # just its first 80 rows.
        with tc.tile_pool(name="ops", bufs=4, space="PSUM") as ops:
            for h in range(HPC):
                for mb in range(2):
                    rows = P if mb == 0 else (NKEPT - P + 2)
                    for nt in range(8):
                        nsl = slice(nt * 512, (nt + 1) * 512)
                        o_ps = ops.tile([P, 512], F32, name="ops_t", tag="ops_t")
                        nc.tensor.matmul(
                            o_ps[:], lhsT=cum_sb[h][:, mb * P:(mb + 1) * P],
                            rhs=wo_sb[h][:, nsl], start=True, stop=True)
                        ob = osb.tile([P, 512], BF16, name="ob", tag="ob")
                        nc.vector.tensor_copy(ob[:rows, :], o_ps[:rows, :])
                        nc.scalar.dma_start(
                            outS[(h * 2 + mb) * P:(h * 2 + mb) * P + rows, nsl],
                            ob[:rows, :])
    nc.compile()
    return nc


def _topk_kept(scores_h):
    """jax.lax.top_k semantics: descending, ties -> lower index."""
    s = scores_h[:-2]
    idx = np.argsort(-s, kind="stable")[:KEEP]
    kept = np.concatenate([idx, [S - 2, S - 1]])
    kept.sort()
    return kept.astype(np.int64)


def kernel(hidden_states, attention_mask, Wq, Wk, Wv, Wo, position_ids):
    x = np.ascontiguousarray(np.asarray(hidden_states, np.float32)[0])  # [S,H]
    Wq = np.asarray(Wq, np.float32)
    Wk = np.asarray(Wk, np.float32)
    Wv = np.asarray(Wv, np.float32)
    Wo = np.asarray(Wo, np.float32)
    pos = np.asarray(position_ids)[0]

    inv = 1.0 / (10000.0 ** (np.arange(0, HD, 2, dtype=np.float32) / HD))
    fr = pos.astype(np.float32)[:, None] * inv
    emb = np.concatenate([fr, fr], -1)
    cosT = np.ascontiguousarray(np.cos(emb).astype(np.float32).T)  # [128, S]
    sinT = np.ascontiguousarray(np.sin(emb).astype(np.float32).T)
    xT = np.ascontiguousarray(x.T)                                 # [H, S]
    scale = np.float32(1.0 / np.sqrt(HD))

    # diagonal-chunk masks: for qt%4 == j, cols (of the 512-wide chunk)
    # beyond j*128+row are masked with -1e9
    row = np.arange(P)[:, None]
    col = np.arange(512)[None, :]
    dmask = np.zeros((4 * P, 512), np.float32)
    for j in range(4):
        dmask[j * P:(j + 1) * P] = np.where(col <= j * P + row, 0.0,
                                            np.float32(-1e9))

    if "p1" not in _cache:
        _cache["p1"] = _build_phase1()
    nc1 = _cache["p1"]

    in_maps = []
    for c in range(NCORES):
        hsl = slice(c * HPC * HD, (c + 1) * HPC * HD)
        wqk = np.concatenate([Wq[hsl, :].T * scale, Wk[hsl, :].T],
                             axis=1).astype(np.float32)
        in_maps.append({
            "xt": xT, "wqk": np.ascontiguousarray(wqk),
            "cos": cosT, "sin": sinT, "dmask": dmask,
        })
    _tr = bool(int(os.environ.get("KTRACE", "0")))
    r1 = run_bass_kernel_spmd(nc1, in_maps, list(range(NCORES)), trace=_tr)
    _cache["exec1"] = r1.exec_time_ns

    # host: top-k, gathers, prefix maps
    x_bf = x.astype(BF)
    Rm = np.triu(np.ones((KPAD, KPAD), np.float32), 1).astype(BF)
    in_maps2, midx = [], []
    for c in range(NCORES):
        scores = r1.results[c]["scores"]
        hsl = slice(c * HPC * HD, (c + 1) * HPC * HD)
        xtkv = np.zeros((H, HPC * KPAD), BF)
        mrows = []
        for h in range(HPC):
            kept = _topk_kept(scores[h])
            xtkv[:, h * KPAD:h * KPAD + NKEPT] = x_bf[kept, :].T
            mrows.append(np.searchsorted(kept, np.arange(S), side="right"))
        midx.append(mrows)
        in_maps2.append({
            "xtk": xtkv,
            "wv": np.ascontiguousarray(Wv[hsl, :].T).astype(BF),
            "wo": np.ascontiguousarray(Wo[:, hsl].T).astype(BF),
            "Rm": Rm,
        })

    if "p2" not in _cache:
        _cache["p2"] = _build_phase2()
    nc2 = _cache["p2"]
    r2 = run_bass_kernel_spmd(nc2, in_maps2, list(range(NCORES)), trace=_tr)
    _cache["exec2"] = r2.exec_time_ns

    # host: expand piecewise-constant rows, sum cores, add exact bias row
    acc = np.zeros((S, H), np.float32)
    for c in range(NCORES):
        oS = np.asarray(r2.results[c]["outS"]).astype(np.float32)
        for h in range(HPC):
            acc += oS[h * KPAD:(h + 1) * KPAD][midx[c][h]]
    xsum = x.astype(np.float64).sum(0)
    vsum = xsum @ Wv.astype(np.float64).T
    bias = (-1e9 * (vsum @ Wo.astype(np.float64).T)).astype(np.float32)
    acc += bias[None, :]
    return acc.reshape(1, S, H)


# revision 32
# speedup vs baseline: 1.0974x; 1.0974x over previous
"""Trainium2 Bass kernel for nn_LlamaAttention_kvcache (sparse H2O attention).

Strategy (8 NeuronCores, tensor-parallel over heads, 4 heads/core):

Phase 1 (device, fp32 storage / fp32r matmuls -- 1 cyc/row at free>=256,
same PE speed as bf16 but TF32-grade mantissa so the per-head top-k
selection matches the fp32 reference exactly):
  q/k projections (scale folded into Wq), RoPE, causal-skipped QK^T
  (only k-chunks at/below the diagonal; diagonal chunk gets an additive
  -1e9 triangular mask), exp with row-accumulation, per-head column
  scores  scores[j] = sum_i exp(aw[i,j]) / r_i  via r^T @ E matmuls.

Host: exact top-k per head (jax.lax.top_k tie semantics), gathers kept
x-rows, builds the prefix matrix map.

Phase 2 (device, bf16): the reference output is
    out = sum_kept (aw+1e9) v  -  1e9 * sum_all v      (per row, exactly)
The (aw+1e9) factor is 1e9 * causal-step + O(10) where the O(10) part
is ~1e-9 of the output scale (expected absmax ~1.8e11), far below fp32
resolution of the accumulated sum, so the device computes the step part:
    po[:, i] = 1e9 * sum_{kept j <= i} v_j
which is piecewise-constant in i with <= NKEPT+1 distinct values.  So
phase 2 computes v for kept tokens, prefix sums via a triangular ones
matmul, and only the <=256 distinct o_proj rows per head:
    outS_h = (1e9 * cumsum(v_h)) @ Wo_h          [256, 4096]
Host expands rows (gather), sums the 8 cores' partials, and adds the
exact -1e9 * (sum_all v) @ Wo rank-1 bias row computed in fp64.
"""

import contextlib
import os
import sys

for p in ("/opt/trn_rl_repo", "/root/.axon_site/_ro/trn_rl_repo"):
    if p not in sys.path:
        sys.path.append(p)

import numpy as np
import ml_dtypes

import concourse.bacc as bacc
import concourse.mybir as mybir
import concourse.tile as tile
from concourse.bass_utils import run_bass_kernel_spmd

F32 = mybir.dt.float32
F32R = mybir.dt.float32r
BF16 = mybir.dt.bfloat16
F16 = mybir.dt.float16
BF = ml_dtypes.bfloat16

P = 128
S = 2048
H = 4096
NH = 32
HD = 128
NCORES = 8
HPC = NH // NCORES          # heads per core = 4
KC = H // P                 # 32 contraction chunks
KEEP = int(0.1 * S)         # 204 top-k heavy hitters
NKEPT = KEEP + 2            # + last-2 local tokens = 206
KPAD = 256                  # padded kept count

_cache = {}


def _r(ap):
    return ap   # operands already declared float32r


def _build_phase1():
    nc = bacc.Bacc("TRN2", target_bir_lowering=False, debug=False,
                   num_devices=NCORES)
    xt = nc.dram_tensor("xt", [H, S], F32R, kind="ExternalInput").ap()
    wqk = nc.dram_tensor("wqk", [H, 2 * HPC * HD], F32R,
                         kind="ExternalInput").ap()
    cosd = nc.dram_tensor("cos", [P, S], F32, kind="ExternalInput").ap()
    sind = nc.dram_tensor("sin", [P, S], F32, kind="ExternalInput").ap()
    dmaskd = nc.dram_tensor("dmask", [P, P], F32,
                            kind="ExternalInput").ap()
    scores_o = nc.dram_tensor("scores", [HPC, S], F32,
                              kind="ExternalOutput").ap()

    with tile.TileContext(nc) as tc, contextlib.ExitStack() as ctx, \
         nc.allow_low_precision(reason="hand-analyzed tf32 score path"):
        # DRAM round-trip buffer for roped q/k (dep-tracked tile):
        # rows [0:512] = qT (4 heads x 128 d), rows [512:1024] = kT.
        dpool = ctx.enter_context(tc.tile_pool(name="dpool", bufs=1,
                                               space="DRAM"))
        qkd = dpool.tile([2 * HPC * HD, S], F32R, name="qkd", tag="qkd")

        # ---- stage A: projections + rope (PE: 32kc x 8 x 4 s-chunks)
        with tc.tile_pool(name="wpool", bufs=1) as wpool, \
             tc.tile_pool(name="cpool", bufs=1) as cpool, \
             tc.tile_pool(name="xpool", bufs=4) as xpool, \
             tc.tile_pool(name="rpool", bufs=2) as rpool, \
             tc.tile_pool(name="stpool", bufs=4) as stpool, \
             tc.tile_pool(name="ppool", bufs=1, space="PSUM") as ppool:
            # weights on the scalar+gpsimd DGE queues so x chunks
            # (sync queue) are not stuck behind 16.8 MB of weight traffic
            w_sb = []
            wq_engines = (nc.scalar, nc.gpsimd)
            for kc in range(KC):
                t = wpool.tile([P, 2 * HPC * HD], F32R, name=f"w{kc}",
                               tag=f"w{kc}")
                wq_engines[kc % 2].dma_start(t[:], wqk[kc * P:(kc + 1) * P, :])
                w_sb.append(t)
            # cos/sin behind the gpsimd weight stream; needed only at ~50us
            cos_sb = cpool.tile([P, S], F32, name="cos_sb", tag="cos")
            sin_sb = cpool.tile([P, S], F32, name="sin_sb", tag="sin")
            nc.gpsimd.dma_start(cos_sb[:], cosd[:, :])
            nc.gpsimd.dma_start(sin_sb[:], sind[:, :])
            IORD = (4, 0, 5, 1, 6, 2, 7, 3)   # k-heads first
            for sq in range(4):
                ssl = slice(sq * 512, (sq + 1) * 512)
                ps = [ppool.tile([P, 512], F32, name=f"pj{i}", tag=f"pj{i}")
                      for i in range(8)]
                for kc in range(KC):
                    xc = xpool.tile([P, 512], F32R, name="xc", tag="xc")
                    nc.sync.dma_start(xc[:], xt[kc * P:(kc + 1) * P, ssl])
                    for i in IORD:
                        nc.tensor.matmul(
                            ps[i][:], lhsT=_r(w_sb[kc][:, i * HD:(i + 1) * HD]),
                            rhs=_r(xc[:]), start=(kc == 0), stop=(kc == KC - 1))
                # first release all 8 PSUM banks with plain copies
                # (alternating DVE/ACT), then do the rope math on the copies
                cps = {}
                for n, i in enumerate(IORD):
                    cp = rpool.tile([P, 512], F32, name="ropecp",
                                    tag=f"ropecp{i}", bufs=2)
                    if n % 2 == 0:
                        nc.vector.tensor_copy(cp[:], ps[i][:])
                    else:
                        nc.scalar.activation(cp[:], ps[i][:],
                                             mybir.ActivationFunctionType.Copy)
                    cps[i] = cp
                for i in IORD:
                    cp = cps[i]
                    m = rpool.tile([P, 512], F32, name="ropem", tag="ropem")
                    nc.vector.tensor_mul(m[:], cp[:], cos_sb[:, ssl])
                    rot = rpool.tile([P, 512], F32, name="roper", tag="roper")
                    # rotate-half on the otherwise-idle scalar engine
                    nc.scalar.activation(rot[0:64, :], cp[64:128, :],
                                         mybir.ActivationFunctionType.Copy,
                                         scale=-1.0)
                    nc.scalar.activation(rot[64:128, :], cp[0:64, :],
                                         mybir.ActivationFunctionType.Copy,
                                         scale=1.0)
                    rs_ = rpool.tile([P, 512], F32, name="ropes", tag="ropes")
                    nc.vector.tensor_mul(rs_[:], rot[:], sin_sb[:, ssl])
                    st = stpool.tile([P, 512], F32R, name="strope", tag="strope")
                    nc.vector.tensor_add(st[:], m[:], rs_[:])
                    nc.sync.dma_start(qkd[i * HD:(i + 1) * HD, ssl], st[:])

        # ---- stage B: causal QK^T + exp + column scores.
        # Two-pass, software-pipelined quarter-head units: pass 1 computes
        # QK chunks + exp (E retained in SBUF) + row sums; pass 2 (one unit
        # behind) does the r^T E score matmuls.  PSUM: aw 2x[128,1024] +
        # one [1,S] score row = 8 banks.
        with tc.tile_pool(name="ktp", bufs=2) as ktp, \
             tc.tile_pool(name="qbp", bufs=2) as qbp, \
             tc.tile_pool(name="epool", bufs=8) as epool, \
             tc.tile_pool(name="vp", bufs=4) as vp, \
             tc.tile_pool(name="rvp", bufs=16) as rvp, \
             tc.tile_pool(name="dmp", bufs=1) as dmp, \
             tc.tile_pool(name="scp", bufs=2) as scp, \
             tc.tile_pool(name="spool", bufs=1, space="PSUM") as spool, \
             tc.tile_pool(name="apool", bufs=3, space="PSUM") as apool:
            dm_sb = dmp.tile([P, P], F32, name="dm", tag="dm")
            nc.sync.dma_start(dm_sb[:], dmaskd[:, :])
            bias5 = dmp.tile([P, 1], F32, name="bias5", tag="bias5")
            nc.vector.memset(bias5[:], -5.0)

            UQT = 4                 # quarter-head pass-1 units
            ESZ = 7424              # sum of exact causal widths in a unit
            state = {}

            def emit_p1(h, q):
                if q == 0:
                    kt = ktp.tile([P, S], F32R, name="kt", tag="kt")
                    qbt = qbp.tile([P, S], F32R, name="qbt", tag="qbt")
                    # column pieces in sq order on the gpsimd queue: piece sq
                    # is ready as soon as stage A's s-chunk sq is written, and
                    # the queue has no write backlog, so early pieces land
                    # immediately (only piece 3 waits on the rope tail)
                    eng = nc.sync if h == 0 else nc.gpsimd
                    for sq in range(4):
                        kssl = slice(sq * 512, (sq + 1) * 512)
                        eng.dma_start(
                            kt[:, kssl],
                            qkd[(HPC + h) * HD:(HPC + h + 1) * HD, kssl])
                        eng.dma_start(
                            qbt[:, kssl],
                            qkd[h * HD:(h + 1) * HD, kssl])
                    state[h] = {"kt": kt, "qbt": qbt, "rinv": {}, "E": {}}
                st_ = state[h]
                E = epool.tile([P, ESZ], F16, name="Eu", tag="Eu")
                st_["E"][q] = E
                eoff = 0
                for qt in range(UQT * q, UQT * q + UQT):
                    W = (qt + 1) * P        # exact causal width
                    rs_list = []
                    for ci, off in enumerate(range(0, W, 1024)):
                        w = min(1024, W - off)
                        aw = apool.tile([P, 1024], F32, name="aw", tag="aw")
                        for sub in range(0, w, 512):
                            sw = min(512, w - sub)
                            nc.tensor.matmul(
                                aw[:, sub:sub + sw],
                                lhsT=_r(st_["qbt"][:, qt * P:(qt + 1) * P]),
                                rhs=_r(st_["kt"][:, off + sub:off + sub + sw]),
                                start=True, stop=True)
                        if off + w == W:   # triangular mask on the last 128
                            nc.vector.tensor_add(aw[:, w - P:w],
                                                 aw[:, w - P:w], dm_sb[:])
                        rs_ = vp.tile([P, 1], F32, name="rsp",
                                      tag=f"rsp{ci}")
                        # exp(aw - 5): keeps E in fp16 range; the e^-5 scale
                        # cancels exactly in scores = (1/rs) * E
                        nc.scalar.activation(E[:, eoff + off:eoff + off + w],
                                             aw[:, :w],
                                             mybir.ActivationFunctionType.Exp,
                                             bias=bias5[:],
                                             accum_out=rs_[:])
                        rs_list.append(rs_)
                    rtot = rs_list[0]
                    for c in range(1, len(rs_list)):
                        nr = vp.tile([P, 1], F32, name="racc", tag=f"racc{c}")
                        nc.vector.tensor_add(nr[:], rtot[:], rs_list[c][:])
                        rtot = nr
                    rinv = rvp.tile([P, 1], F16, name="rinv", tag="rinv")
                    nc.vector.reciprocal(rinv[:], rtot[:])
                    st_["rinv"][qt] = rinv
                    eoff += W

            def emit_p2(h, half):
                # score matmuls for column half [1024*half, 1024*(half+1))
                st_ = state[h]
                base = 1024 * half
                sc = spool.tile([1, 1024], F32, name="scps", tag="scps")
                if half == 0:
                    st_["scsb"] = scp.tile([1, S], F32, name="scsb",
                                           tag="scsb")
                first_qt = 8 * half
                for qt in range(first_qt, 16):
                    W = (qt + 1) * P
                    w = min(1024, W - base)
                    if w <= 0:
                        continue
                    eoff = sum((t + 1) * P
                               for t in range(UQT * (qt // UQT), qt))
                    E = st_["E"][qt // UQT]
                    rinv = st_["rinv"][qt]
                    for sub in range(0, w, 512):
                        sw = min(512, w - sub)
                        nc.tensor.matmul(
                            sc[:, sub:sub + sw], lhsT=rinv[:],
                            rhs=E[:, eoff + base + sub:
                                    eoff + base + sub + sw],
                            start=(qt == (base + sub) // P),
                            stop=(qt == 15))
                nc.vector.tensor_copy(st_["scsb"][:, base:base + 1024],
                                      sc[:])
                if half == 1:
                    nc.gpsimd.dma_start(scores_o[h:h + 1, :], st_["scsb"][:])

            for h in range(HPC):
                for q in range(4):
                    emit_p1(h, q)
                    if h > 0 and q == 1:
                        emit_p2(h - 1, 0)
                    if h > 0 and q == 3:
                        emit_p2(h - 1, 1)
            emit_p2(HPC - 1, 0)
            emit_p2(HPC - 1, 1)
    nc.compile()
    return nc


def _build_phase2():
    nc = bacc.Bacc("TRN2", target_bir_lowering=False, debug=False,
                   num_devices=NCORES)
    xtk = nc.dram_tensor("xtk", [H, HPC * KPAD], BF16,
                         kind="ExternalInput").ap()
    wv = nc.dram_tensor("wv", [H, HPC * HD], BF16, kind="ExternalInput").ap()
    wo = nc.dram_tensor("wo", [HPC * HD, H], BF16, kind="ExternalInput").ap()
    Rmd = nc.dram_tensor("Rm", [KPAD, KPAD], BF16, kind="ExternalInput").ap()
    outS = nc.dram_tensor("outS", [HPC * KPAD, H], BF16,
                          kind="ExternalOutput").ap()

    with tile.TileContext(nc) as tc, contextlib.ExitStack() as ctx:
        wvp = ctx.enter_context(tc.tile_pool(name="wvp", bufs=1))
        wop = ctx.enter_context(tc.tile_pool(name="wop", bufs=1))
        xkp = ctx.enter_context(tc.tile_pool(name="xkp", bufs=6))
        rp = ctx.enter_context(tc.tile_pool(name="rp", bufs=1))
        vsb = ctx.enter_context(tc.tile_pool(name="vsb", bufs=1))
        csb = ctx.enter_context(tc.tile_pool(name="csb", bufs=1))
        osb = ctx.enter_context(tc.tile_pool(name="osb", bufs=6))

        R_sb = [rp.tile([P, KPAD], BF16, name=f"R{t}", tag=f"R{t}")
                for t in range(2)]
        for t in range(2):
            nc.scalar.dma_start(R_sb[t][:], Rmd[t * P:(t + 1) * P, :])
        # wv on the scalar queue so xtk chunks (sync queue) start immediately
        wv_sb = []
        for kc in range(KC):
            t = wvp.tile([P, HPC * HD], BF16, name=f"wv{kc}", tag=f"wv{kc}")
            nc.scalar.dma_start(t[:], wv[kc * P:(kc + 1) * P, :])
            wv_sb.append(t)

        # v projection of kept tokens: v_sb[h][t] = [128 kept, 128 d] bf16
        v_sb = [[vsb.tile([P, HD], BF16, name=f"vsb{h}_{t}", tag=f"vsb{h}_{t}")
                 for t in range(2)] for h in range(HPC)]
        with tc.tile_pool(name="vps", bufs=1, space="PSUM") as vps:
            v_ps = [[vps.tile([P, HD], F32, name=f"vps{h}_{t}",
                              tag=f"vps{h}_{t}")
                     for t in range(2)] for h in range(HPC)]
            for kc in range(KC):
                xk = xkp.tile([P, HPC * KPAD], BF16, name="xk", tag="xk")
                nc.sync.dma_start(xk[:], xtk[kc * P:(kc + 1) * P, :])
                for h in range(HPC):
                    for t in range(2):
                        nc.tensor.matmul(
                            v_ps[h][t][:],
                            lhsT=xk[:, h * KPAD + t * P:h * KPAD + (t + 1) * P],
                            rhs=wv_sb[kc][:, h * HD:(h + 1) * HD],
                            start=(kc == 0), stop=(kc == KC - 1))
            for h in range(HPC):
                for t in range(2):
                    nc.vector.tensor_copy(v_sb[h][t][:], v_ps[h][t][:])

        # wo loads emitted after the v-proj stream so they don't delay it
        wo_sb = [wop.tile([P, H], BF16, name=f"wo{h}", tag=f"wo{h}")
                 for h in range(HPC)]
        for h in range(HPC):
            nc.gpsimd.dma_start(wo_sb[h][:], wo[h * P:(h + 1) * P, :])

        # prefix sums over sorted kept order: cumT[h] = [128 d, 256 m] bf16,
        # scaled by 1e9 on the PSUM->SBUF copy.
        cum_sb = [csb.tile([P, KPAD], BF16, name=f"cum{h}", tag=f"cum{h}")
                  for h in range(HPC)]
        with tc.tile_pool(name="cps", bufs=1, space="PSUM") as cps:
            for h in range(HPC):
                cum_ps = cps.tile([P, KPAD], F32, name="cumps", tag=f"cps{h}")
                for t in range(2):
                    nc.tensor.matmul(cum_ps[:], lhsT=v_sb[h][t][:],
                                     rhs=R_sb[t][:],
                                     start=(t == 0), stop=(t == 1))
                nc.scalar.activation(cum_sb[h][:], cum_ps[:],
                                     mybir.ActivationFunctionType.Copy,
                                     scale=1e9)

        # distinct o_proj rows: outS[h*256+m, :] = cumT[h][:, m] @ wo_h
        # only m <= NKEPT=206 is ever gathered, so the mb=1 block writes
        # just its first 80 rows.
        with tc.tile_pool(name="ops", bufs=4, space="PSUM") as ops:
            for h in range(HPC):
                for mb in range(2):
                    rows = P if mb == 0 else (NKEPT - P + 2)
                    for nt in range(8):
                        nsl = slice(nt * 512, (nt + 1) * 512)
                        o_ps = ops.tile([P, 512], F32, name="ops_t", tag="ops_t")
                        nc.tensor.matmul(
                            o_ps[:], lhsT=cum_sb[h][:, mb * P:(mb + 1) * P],
                            rhs=wo_sb[h][:, nsl], start=True, stop=True)
                        ob = osb.tile([P, 512], BF16, name="ob", tag="ob")
                        nc.vector.tensor_copy(ob[:rows, :], o_ps[:rows, :])
                        nc.scalar.dma_start(
                            outS[(h * 2 + mb) * P:(h * 2 + mb) * P + rows, nsl],
                            ob[:rows, :])
    nc.compile()
    return nc


def _topk_kept(scores_h):
    """jax.lax.top_k semantics: descending, ties -> lower index."""
    s = scores_h[:-2]
    idx = np.argsort(-s, kind="stable")[:KEEP]
    kept = np.concatenate([idx, [S - 2, S - 1]])
    kept.sort()
    return kept.astype(np.int64)


def kernel(hidden_states, attention_mask, Wq, Wk, Wv, Wo, position_ids):
    x = np.ascontiguousarray(np.asarray(hidden_states, np.float32)[0])  # [S,H]
    Wq = np.asarray(Wq, np.float32)
    Wk = np.asarray(Wk, np.float32)
    Wv = np.asarray(Wv, np.float32)
    Wo = np.asarray(Wo, np.float32)
    pos = np.asarray(position_ids)[0]

    inv = 1.0 / (10000.0 ** (np.arange(0, HD, 2, dtype=np.float32) / HD))
    fr = pos.astype(np.float32)[:, None] * inv
    emb = np.concatenate([fr, fr], -1)
    cosT = np.ascontiguousarray(np.cos(emb).astype(np.float32).T)  # [128, S]
    sinT = np.ascontiguousarray(np.sin(emb).astype(np.float32).T)
    xT = np.ascontiguousarray(x.T)                                 # [H, S]
    scale = np.float32(1.0 / np.sqrt(HD))

    # diagonal-chunk masks: for qt%4 == j, cols (of the 512-wide chunk)
    # beyond j*128+row are masked with -1e9
    row = np.arange(P)[:, None]
    col = np.arange(P)[None, :]
    dmask = np.where(col <= row, 0.0, np.float32(-1e9)).astype(np.float32)

    if "p1" not in _cache:
        _cache["p1"] = _build_phase1()
    nc1 = _cache["p1"]

    in_maps = []
    for c in range(NCORES):
        hsl = slice(c * HPC * HD, (c + 1) * HPC * HD)
        wqk = np.concatenate([Wq[hsl, :].T * scale, Wk[hsl, :].T],
                             axis=1).astype(np.float32)
        in_maps.append({
            "xt": xT, "wqk": np.ascontiguousarray(wqk),
            "cos": cosT, "sin": sinT, "dmask": dmask,
        })
    _tr = bool(int(os.environ.get("KTRACE", "0")))
    r1 = run_bass_kernel_spmd(nc1, in_maps, list(range(NCORES)), trace=_tr)
    _cache["exec1"] = r1.exec_time_ns

    # host: top-k, gathers, prefix maps
    x_bf = x.astype(BF)
    Rm = np.triu(np.ones((KPAD, KPAD), np.float32), 1).astype(BF)
    in_maps2, midx = [], []
    for c in range(NCORES):
        scores = r1.results[c]["scores"]
        hsl = slice(c * HPC * HD, (c + 1) * HPC * HD)
        xtkv = np.zeros((H, HPC * KPAD), BF)
        mrows = []
        for h in range(HPC):
            kept = _topk_kept(scores[h])
            xtkv[:, h * KPAD:h * KPAD + NKEPT] = x_bf[kept, :].T
            mrows.append(np.searchsorted(kept, np.arange(S), side="right"))
        midx.append(mrows)
        in_maps2.append({
            "xtk": xtkv,
            "wv": np.ascontiguousarray(Wv[hsl, :].T).astype(BF),
            "wo": np.ascontiguousarray(Wo[:, hsl].T).astype(BF),
            "Rm": Rm,
        })

    if "p2" not in _cache:
        _cache["p2"] = _build_phase2()
    nc2 = _cache["p2"]
    r2 = run_bass_kernel_spmd(nc2, in_maps2, list(range(NCORES)), trace=_tr)
    _cache["exec2"] = r2.exec_time_ns

    # host: expand piecewise-constant rows, sum cores, add exact bias row
    acc = np.zeros((S, H), np.float32)
    for c in range(NCORES):
        oS = np.asarray(r2.results[c]["outS"]).astype(np.float32)
        for h in range(HPC):
            acc += oS[h * KPAD:(h + 1) * KPAD][midx[c][h]]
    xsum = x.astype(np.float64).sum(0)
    vsum = xsum @ Wv.astype(np.float64).T
    bias = (-1e9 * (vsum @ Wo.astype(np.float64).T)).astype(np.float32)
    acc += bias[None, :]
    return acc.reshape(1, S, H)


# revision 39
# speedup vs baseline: 1.1334x; 1.0328x over previous
"""Trainium2 Bass kernel for nn_LlamaAttention_kvcache (sparse H2O attention).

Strategy (8 NeuronCores, tensor-parallel over heads, 4 heads/core):

Phase 1 (device, fp32 storage / fp32r matmuls -- 1 cyc/row at free>=256,
same PE speed as bf16 but TF32-grade mantissa so the per-head top-k
selection matches the fp32 reference exactly):
  q/k projections (scale folded into Wq), RoPE, causal-skipped QK^T
  (only k-chunks at/below the diagonal; diagonal chunk gets an additive
  -1e9 triangular mask), exp with row-accumulation, per-head column
  scores  scores[j] = sum_i exp(aw[i,j]) / r_i  via r^T @ E matmuls.

Host: exact top-k per head (jax.lax.top_k tie semantics), gathers kept
x-rows, builds the prefix matrix map.

Phase 2 (device, bf16): the reference output is
    out = sum_kept (aw+1e9) v  -  1e9 * sum_all v      (per row, exactly)
The (aw+1e9) factor is 1e9 * causal-step + O(10) where the O(10) part
is ~1e-9 of the output scale (expected absmax ~1.8e11), far below fp32
resolution of the accumulated sum, so the device computes the step part:
    po[:, i] = 1e9 * sum_{kept j <= i} v_j
which is piecewise-constant in i with <= NKEPT+1 distinct values.  So
phase 2 computes v for kept tokens, prefix sums via a triangular ones
matmul, and only the <=256 distinct o_proj rows per head:
    outS_h = (1e9 * cumsum(v_h)) @ Wo_h          [256, 4096]
Host expands rows (gather), sums the 8 cores' partials, and adds the
exact -1e9 * (sum_all v) @ Wo rank-1 bias row computed in fp64.
"""

import contextlib
import os
import sys

for p in ("/opt/trn_rl_repo", "/root/.axon_site/_ro/trn_rl_repo"):
    if p not in sys.path:
        sys.path.append(p)

import numpy as np
import ml_dtypes

import concourse.bacc as bacc
import concourse.mybir as mybir
import concourse.tile as tile
from concourse.bass_utils import run_bass_kernel_spmd
from concourse.tile import add_dep_helper

F32 = mybir.dt.float32
F32R = mybir.dt.float32r
BF16 = mybir.dt.bfloat16
F16 = mybir.dt.float16
BF = ml_dtypes.bfloat16

P = 128
S = 2048
H = 4096
NH = 32
HD = 128
NCORES = 8
HPC = NH // NCORES          # heads per core = 4
KC = H // P                 # 32 contraction chunks
KEEP = int(0.1 * S)         # 204 top-k heavy hitters
NKEPT = KEEP + 2            # + last-2 local tokens = 206
KPAD = 256                  # padded kept count

_cache = {}


def _r(ap):
    return ap   # operands already declared float32r


def _build_phase1():
    nc = bacc.Bacc("TRN2", target_bir_lowering=False, debug=False,
                   num_devices=NCORES)
    xt = nc.dram_tensor("xt", [H, S], F32R, kind="ExternalInput").ap()
    wqk = nc.dram_tensor("wqk", [H, 2 * HPC * HD], F32R,
                         kind="ExternalInput").ap()
    cosd = nc.dram_tensor("cos", [P, S], F32, kind="ExternalInput").ap()
    sind = nc.dram_tensor("sin", [P, S], F32, kind="ExternalInput").ap()
    dmaskd = nc.dram_tensor("dmask", [P, P], F32,
                            kind="ExternalInput").ap()
    scores_o = nc.dram_tensor("scores", [HPC, S], F32,
                              kind="ExternalOutput").ap()

    with tile.TileContext(nc) as tc, contextlib.ExitStack() as ctx, \
         nc.allow_low_precision(reason="hand-analyzed tf32 score path"):
        # DRAM round-trip buffer for roped q/k (dep-tracked tile):
        # rows [0:512] = qT (4 heads x 128 d), rows [512:1024] = kT.
        dpool = ctx.enter_context(tc.tile_pool(name="dpool", bufs=1,
                                               space="DRAM"))
        qkd = dpool.tile([2 * HPC * HD, S], F32R, name="qkd", tag="qkd")

        # ---- stage A: projections + rope (PE: 32kc x 8 x 4 s-chunks)
        with tc.tile_pool(name="wpool", bufs=1) as wpool, \
             tc.tile_pool(name="cpool", bufs=1) as cpool, \
             tc.tile_pool(name="xpool", bufs=4) as xpool, \
             tc.tile_pool(name="rpool", bufs=2) as rpool, \
             tc.tile_pool(name="stpool", bufs=4) as stpool, \
             tc.tile_pool(name="ppool", bufs=1, space="PSUM") as ppool:
            # weights on the scalar+gpsimd DGE queues so x chunks
            # (sync queue) are not stuck behind 16.8 MB of weight traffic
            w_sb = []
            wq_engines = (nc.scalar, nc.gpsimd)
            for kc in range(KC):
                t = wpool.tile([P, 2 * HPC * HD], F32R, name=f"w{kc}",
                               tag=f"w{kc}")
                wq_engines[kc % 2].dma_start(t[:], wqk[kc * P:(kc + 1) * P, :])
                w_sb.append(t)
            # cos/sin behind the gpsimd weight stream; needed only at ~50us
            cos_sb = cpool.tile([P, S], F32, name="cos_sb", tag="cos")
            sin_sb = cpool.tile([P, S], F32, name="sin_sb", tag="sin")
            nc.gpsimd.dma_start(cos_sb[:], cosd[:, :])
            nc.gpsimd.dma_start(sin_sb[:], sind[:, :])
            IORD = (4, 0, 5, 1, 6, 2, 7, 3)   # k-heads first
            for sq in range(4):
                ssl = slice(sq * 512, (sq + 1) * 512)
                ps = [ppool.tile([P, 512], F32, name=f"pj{i}", tag=f"pj{i}")
                      for i in range(8)]
                for kc in range(KC):
                    xc = xpool.tile([P, 512], F32R, name="xc", tag="xc")
                    nc.sync.dma_start(xc[:], xt[kc * P:(kc + 1) * P, ssl])
                    for i in IORD:
                        nc.tensor.matmul(
                            ps[i][:], lhsT=_r(w_sb[kc][:, i * HD:(i + 1) * HD]),
                            rhs=_r(xc[:]), start=(kc == 0), stop=(kc == KC - 1))
                # first release all 8 PSUM banks with plain copies
                # (alternating DVE/ACT), then do the rope math on the copies
                cps = {}
                for n, i in enumerate(IORD):
                    cp = rpool.tile([P, 512], F32, name="ropecp",
                                    tag=f"ropecp{i}", bufs=2)
                    if n % 2 == 0:
                        nc.vector.tensor_copy(cp[:], ps[i][:])
                    else:
                        nc.scalar.activation(cp[:], ps[i][:],
                                             mybir.ActivationFunctionType.Copy)
                    cps[i] = cp
                for i in IORD:
                    cp = cps[i]
                    m = rpool.tile([P, 512], F32, name="ropem", tag="ropem")
                    nc.vector.tensor_mul(m[:], cp[:], cos_sb[:, ssl])
                    rot = rpool.tile([P, 512], F32, name="roper", tag="roper")
                    # rotate-half on the otherwise-idle scalar engine
                    nc.scalar.activation(rot[0:64, :], cp[64:128, :],
                                         mybir.ActivationFunctionType.Copy,
                                         scale=-1.0)
                    nc.scalar.activation(rot[64:128, :], cp[0:64, :],
                                         mybir.ActivationFunctionType.Copy,
                                         scale=1.0)
                    rs_ = rpool.tile([P, 512], F32, name="ropes", tag="ropes")
                    nc.vector.tensor_mul(rs_[:], rot[:], sin_sb[:, ssl])
                    st = stpool.tile([P, 512], F32R, name="strope", tag="strope")
                    nc.vector.tensor_add(st[:], m[:], rs_[:])
                    nc.sync.dma_start(qkd[i * HD:(i + 1) * HD, ssl], st[:])

        # ---- stage B: causal QK^T + exp + column scores.
        # Two-pass, software-pipelined quarter-head units: pass 1 computes
        # QK chunks + exp (E retained in SBUF) + row sums; pass 2 (one unit
        # behind) does the r^T E score matmuls.  PSUM: aw 2x[128,1024] +
        # one [1,S] score row = 8 banks.
        with tc.tile_pool(name="ktp", bufs=2) as ktp, \
             tc.tile_pool(name="qbp", bufs=2) as qbp, \
             tc.tile_pool(name="epool", bufs=8) as epool, \
             tc.tile_pool(name="vp", bufs=4) as vp, \
             tc.tile_pool(name="rvp", bufs=16) as rvp, \
             tc.tile_pool(name="dmp", bufs=1) as dmp, \
             tc.tile_pool(name="scp", bufs=2) as scp, \
             tc.tile_pool(name="spool", bufs=1, space="PSUM") as spool, \
             tc.tile_pool(name="apool", bufs=3, space="PSUM") as apool:
            dm_sb = dmp.tile([P, P], F32, name="dm", tag="dm")
            nc.sync.dma_start(dm_sb[:], dmaskd[:, :])
            bias5 = dmp.tile([P, 1], F32, name="bias5", tag="bias5")
            nc.vector.memset(bias5[:], -5.0)

            UQT = 4                 # quarter-head pass-1 units
            ESZ = 7424              # sum of exact causal widths in a unit
            state = {}

            def emit_p1(h, q):
                if q == 0:
                    kt = ktp.tile([P, S], F32R, name="kt", tag="kt")
                    qbt = qbp.tile([P, S], F32R, name="qbt", tag="qbt")
                    # column pieces in sq order on the gpsimd queue: piece sq
                    # is ready as soon as stage A's s-chunk sq is written, and
                    # the queue has no write backlog, so early pieces land
                    # immediately (only piece 3 waits on the rope tail)
                    eng = nc.sync if h == 0 else nc.gpsimd
                    for sq in range(4):
                        kssl = slice(sq * 512, (sq + 1) * 512)
                        eng.dma_start(
                            kt[:, kssl],
                            qkd[(HPC + h) * HD:(HPC + h + 1) * HD, kssl])
                        eng.dma_start(
                            qbt[:, kssl],
                            qkd[h * HD:(h + 1) * HD, kssl])
                    state[h] = {"kt": kt, "qbt": qbt, "rinv": {}, "E": {}}
                st_ = state[h]
                E = epool.tile([P, ESZ], F16, name="Eu", tag="Eu")
                st_["E"][q] = E
                eoff = 0
                for qt in range(UQT * q, UQT * q + UQT):
                    W = (qt + 1) * P        # exact causal width
                    rs_list = []
                    for ci, off in enumerate(range(0, W, 1024)):
                        w = min(1024, W - off)
                        aw = apool.tile([P, 1024], F32, name="aw", tag="aw")
                        for sub in range(0, w, 512):
                            sw = min(512, w - sub)
                            nc.tensor.matmul(
                                aw[:, sub:sub + sw],
                                lhsT=_r(st_["qbt"][:, qt * P:(qt + 1) * P]),
                                rhs=_r(st_["kt"][:, off + sub:off + sub + sw]),
                                start=True, stop=True)
                        if off + w == W:   # triangular mask on the last 128
                            nc.vector.tensor_add(aw[:, w - P:w],
                                                 aw[:, w - P:w], dm_sb[:])
                        rs_ = vp.tile([P, 1], F32, name="rsp",
                                      tag=f"rsp{ci}")
                        # exp(aw - 5): keeps E in fp16 range; the e^-5 scale
                        # cancels exactly in scores = (1/rs) * E
                        nc.scalar.activation(E[:, eoff + off:eoff + off + w],
                                             aw[:, :w],
                                             mybir.ActivationFunctionType.Exp,
                                             bias=bias5[:],
                                             accum_out=rs_[:])
                        rs_list.append(rs_)
                    rtot = rs_list[0]
                    for c in range(1, len(rs_list)):
                        nr = vp.tile([P, 1], F32, name="racc", tag=f"racc{c}")
                        nc.vector.tensor_add(nr[:], rtot[:], rs_list[c][:])
                        rtot = nr
                    rinv = rvp.tile([P, 1], F16, name="rinv", tag="rinv")
                    nc.vector.reciprocal(rinv[:], rtot[:])
                    st_["rinv"][qt] = rinv
                    eoff += W

            def emit_p2(h, half):
                # score matmuls for column half [1024*half, 1024*(half+1))
                st_ = state[h]
                base = 1024 * half
                sc = spool.tile([1, 1024], F32, name="scps", tag="scps")
                if half == 0:
                    st_["scsb"] = scp.tile([1, S], F32, name="scsb",
                                           tag="scsb")
                first_qt = 8 * half
                for qt in range(first_qt, 16):
                    W = (qt + 1) * P
                    w = min(1024, W - base)
                    if w <= 0:
                        continue
                    eoff = sum((t + 1) * P
                               for t in range(UQT * (qt // UQT), qt))
                    E = st_["E"][qt // UQT]
                    rinv = st_["rinv"][qt]
                    for sub in range(0, w, 512):
                        sw = min(512, w - sub)
                        nc.tensor.matmul(
                            sc[:, sub:sub + sw], lhsT=rinv[:],
                            rhs=E[:, eoff + base + sub:
                                    eoff + base + sub + sw],
                            start=(qt == (base + sub) // P),
                            stop=(qt == 15))
                nc.vector.tensor_copy(st_["scsb"][:, base:base + 1024],
                                      sc[:])
                if half == 1:
                    nc.gpsimd.dma_start(scores_o[h:h + 1, :], st_["scsb"][:])

            for h in range(HPC):
                for q in range(4):
                    emit_p1(h, q)
                    if h > 0 and q == 1:
                        emit_p2(h - 1, 0)
                    if h > 0 and q == 3:
                        emit_p2(h - 1, 1)
            emit_p2(HPC - 1, 0)
            emit_p2(HPC - 1, 1)
    nc.compile()
    return nc


def _build_phase2():
    nc = bacc.Bacc("TRN2", target_bir_lowering=False, debug=False,
                   num_devices=NCORES)
    # packed: row p, col kc*W+c holds original row kc*128+p, col c
    xtk = nc.dram_tensor("xtk", [P, KC * HPC * KPAD], BF16,
                         kind="ExternalInput").ap()
    wv = nc.dram_tensor("wv", [P, KC * HPC * HD], BF16,
                        kind="ExternalInput").ap()
    wo = nc.dram_tensor("wo", [HPC * HD, H], BF16, kind="ExternalInput").ap()
    Rmd = nc.dram_tensor("Rm", [KPAD, KPAD], BF16, kind="ExternalInput").ap()
    outS = nc.dram_tensor("outS", [HPC * KPAD, H], BF16,
                          kind="ExternalOutput").ap()

    with tile.TileContext(nc) as tc, contextlib.ExitStack() as ctx:
        wvp = ctx.enter_context(tc.tile_pool(name="wvp", bufs=1))
        wop = ctx.enter_context(tc.tile_pool(name="wop", bufs=1))
        xkp = ctx.enter_context(tc.tile_pool(name="xkp", bufs=1))
        rp = ctx.enter_context(tc.tile_pool(name="rp", bufs=1))
        vsb = ctx.enter_context(tc.tile_pool(name="vsb", bufs=1))
        csb = ctx.enter_context(tc.tile_pool(name="csb", bufs=1))
        osb = ctx.enter_context(tc.tile_pool(name="osb", bufs=6))

        # wv packed into one resident tile (2 big DMAs on the scalar queue)
        WVW = HPC * HD
        wv_big = wvp.tile([P, KC * WVW], BF16, name="wv_big", tag="wv_big")
        for hh in range(2):
            csl = slice(hh * KC * WVW // 2, (hh + 1) * KC * WVW // 2)
            nc.scalar.dma_start(wv_big[:, csl], wv[:, csl])
        wv_sb = [wv_big[:, kc * WVW:(kc + 1) * WVW] for kc in range(KC)]
        R_sb = [rp.tile([P, KPAD], BF16, name=f"R{t}", tag=f"R{t}")
                for t in range(2)]
        for t in range(2):
            nc.scalar.dma_start(R_sb[t][:], Rmd[t * P:(t + 1) * P, :])

        # v projection of kept tokens: v_sb[h][t] = [128 kept, 128 d] bf16
        v_sb = [[vsb.tile([P, HD], BF16, name=f"vsb{h}_{t}", tag=f"vsb{h}_{t}")
                 for t in range(2)] for h in range(HPC)]
        with tc.tile_pool(name="vps", bufs=1, space="PSUM") as vps:
            v_ps = [[vps.tile([P, HD], F32, name=f"vps{h}_{t}",
                              tag=f"vps{h}_{t}")
                     for t in range(2)] for h in range(HPC)]
            XKW = HPC * KPAD
            xk_big = xkp.tile([P, KC * XKW], BF16, name="xk_big",
                              tag="xk_big")
            xk_insts = []
            for qq in range(4):
                csl = slice(qq * KC * XKW // 4, (qq + 1) * KC * XKW // 4)
                xk_insts.append(nc.sync.dma_start(xk_big[:, csl],
                                                  xtk[:, csl]))
            for kc in range(KC):
                xk = xk_big[:, kc * XKW:(kc + 1) * XKW]
                for h in range(HPC):
                    for t in range(2):
                        nc.tensor.matmul(
                            v_ps[h][t][:],
                            lhsT=xk[:, h * KPAD + t * P:h * KPAD + (t + 1) * P],
                            rhs=wv_sb[kc][:, h * HD:(h + 1) * HD],
                            start=(kc == 0), stop=(kc == KC - 1))
            for h in range(HPC):
                for t in range(2):
                    nc.vector.tensor_copy(v_sb[h][t][:], v_ps[h][t][:])

        # wo loads emitted after the v-proj stream so they don't delay it
        wo_sb = [wop.tile([P, H], BF16, name=f"wo{h}", tag=f"wo{h}")
                 for h in range(HPC)]
        for h in range(HPC):
            wo_i = nc.gpsimd.dma_start(wo_sb[h][:], wo[h * P:(h + 1) * P, :])
            # hold the 4 MB wo stream until the v-proj input stream is
            # mostly through the DMA engines
            add_dep_helper(xk_insts[2].ins, wo_i.ins,
                           reason="wo transfers after v-proj inputs")

        # prefix sums over sorted kept order: cumT[h] = [128 d, 256 m] bf16,
        # scaled by 1e9 on the PSUM->SBUF copy.
        cum_sb = [csb.tile([P, KPAD], BF16, name=f"cum{h}", tag=f"cum{h}")
                  for h in range(HPC)]
        with tc.tile_pool(name="cps", bufs=1, space="PSUM") as cps:
            for h in range(HPC):
                cum_ps = cps.tile([P, KPAD], F32, name="cumps", tag=f"cps{h}")
                for t in range(2):
                    nc.tensor.matmul(cum_ps[:], lhsT=v_sb[h][t][:],
                                     rhs=R_sb[t][:],
                                     start=(t == 0), stop=(t == 1))
                nc.scalar.activation(cum_sb[h][:], cum_ps[:],
                                     mybir.ActivationFunctionType.Copy,
                                     scale=1e9)

        # distinct o_proj rows: outS[h*256+m, :] = cumT[h][:, m] @ wo_h
        # only m <= NKEPT=206 is ever gathered, so the mb=1 block writes
        # just its first 80 rows.
        with tc.tile_pool(name="ops", bufs=4, space="PSUM") as ops:
            for h in range(HPC):
                for mb in range(2):
                    rows = P if mb == 0 else (NKEPT - P + 2)
                    ob = osb.tile([P, H], BF16, name="ob", tag="ob")
                    for nt in range(8):
                        nsl = slice(nt * 512, (nt + 1) * 512)
                        o_ps = ops.tile([P, 512], F32, name="ops_t", tag="ops_t")
                        nc.tensor.matmul(
                            o_ps[:], lhsT=cum_sb[h][:, mb * P:(mb + 1) * P],
                            rhs=wo_sb[h][:, nsl], start=True, stop=True)
                        if nt % 2 == 0:
                            nc.vector.tensor_copy(ob[:rows, nsl],
                                                  o_ps[:rows, :])
                        else:
                            nc.scalar.activation(
                                ob[:rows, nsl], o_ps[:rows, :],
                                mybir.ActivationFunctionType.Copy)
                    nc.sync.dma_start(
                        outS[(h * 2 + mb) * P:(h * 2 + mb) * P + rows, :],
                        ob[:rows, :])
    nc.compile()
    return nc


def _topk_kept(scores_h):
    """jax.lax.top_k semantics: descending, ties -> lower index."""
    s = scores_h[:-2]
    idx = np.argsort(-s, kind="stable")[:KEEP]
    kept = np.concatenate([idx, [S - 2, S - 1]])
    kept.sort()
    return kept.astype(np.int64)


def kernel(hidden_states, attention_mask, Wq, Wk, Wv, Wo, position_ids):
    x = np.ascontiguousarray(np.asarray(hidden_states, np.float32)[0])  # [S,H]
    Wq = np.asarray(Wq, np.float32)
    Wk = np.asarray(Wk, np.float32)
    Wv = np.asarray(Wv, np.float32)
    Wo = np.asarray(Wo, np.float32)
    pos = np.asarray(position_ids)[0]

    inv = 1.0 / (10000.0 ** (np.arange(0, HD, 2, dtype=np.float32) / HD))
    fr = pos.astype(np.float32)[:, None] * inv
    emb = np.concatenate([fr, fr], -1)
    cosT = np.ascontiguousarray(np.cos(emb).astype(np.float32).T)  # [128, S]
    sinT = np.ascontiguousarray(np.sin(emb).astype(np.float32).T)
    xT = np.ascontiguousarray(x.T)                                 # [H, S]
    scale = np.float32(1.0 / np.sqrt(HD))

    # diagonal-chunk masks: for qt%4 == j, cols (of the 512-wide chunk)
    # beyond j*128+row are masked with -1e9
    row = np.arange(P)[:, None]
    col = np.arange(P)[None, :]
    dmask = np.where(col <= row, 0.0, np.float32(-1e9)).astype(np.float32)

    if "p1" not in _cache:
        _cache["p1"] = _build_phase1()
    nc1 = _cache["p1"]

    in_maps = []
    for c in range(NCORES):
        hsl = slice(c * HPC * HD, (c + 1) * HPC * HD)
        wqk = np.concatenate([Wq[hsl, :].T * scale, Wk[hsl, :].T],
                             axis=1).astype(np.float32)
        in_maps.append({
            "xt": xT, "wqk": np.ascontiguousarray(wqk),
            "cos": cosT, "sin": sinT, "dmask": dmask,
        })
    _tr = bool(int(os.environ.get("KTRACE", "0")))
    r1 = run_bass_kernel_spmd(nc1, in_maps, list(range(NCORES)), trace=_tr)
    _cache["exec1"] = r1.exec_time_ns

    # host: top-k, gathers, prefix maps
    x_bf = x.astype(BF)
    Rm = np.triu(np.ones((KPAD, KPAD), np.float32), 1).astype(BF)
    in_maps2, midx = [], []
    for c in range(NCORES):
        scores = r1.results[c]["scores"]
        hsl = slice(c * HPC * HD, (c + 1) * HPC * HD)
        xtkv = np.zeros((H, HPC * KPAD), BF)
        mrows = []
        for h in range(HPC):
            kept = _topk_kept(scores[h])
            xtkv[:, h * KPAD:h * KPAD + NKEPT] = x_bf[kept, :].T
            mrows.append(np.searchsorted(kept, np.arange(S), side="right"))
        midx.append(mrows)
        xtkp = np.ascontiguousarray(
            xtkv.reshape(KC, P, HPC * KPAD).transpose(1, 0, 2)
                .reshape(P, KC * HPC * KPAD))
        wvp_ = np.ascontiguousarray(
            Wv[hsl, :].T.astype(BF).reshape(KC, P, HPC * HD)
            .transpose(1, 0, 2).reshape(P, KC * HPC * HD))
        in_maps2.append({
            "xtk": xtkp,
            "wv": wvp_,
            "wo": np.ascontiguousarray(Wo[:, hsl].T).astype(BF),
            "Rm": Rm,
        })

    if "p2" not in _cache:
        _cache["p2"] = _build_phase2()
    nc2 = _cache["p2"]
    r2 = run_bass_kernel_spmd(nc2, in_maps2, list(range(NCORES)), trace=_tr)
    _cache["exec2"] = r2.exec_time_ns

    # host: expand piecewise-constant rows, sum cores, add exact bias row
    acc = np.zeros((S, H), np.float32)
    for c in range(NCORES):
        oS = np.asarray(r2.results[c]["outS"]).astype(np.float32)
        for h in range(HPC):
            acc += oS[h * KPAD:(h + 1) * KPAD][midx[c][h]]
    xsum = x.astype(np.float64).sum(0)
    vsum = xsum @ Wv.astype(np.float64).T
    bias = (-1e9 * (vsum @ Wo.astype(np.float64).T)).astype(np.float32)
    acc += bias[None, :]
    return acc.reshape(1, S, H)


# revision 46
# speedup vs baseline: 1.1621x; 1.0253x over previous
"""Trainium2 Bass kernel for nn_LlamaAttention_kvcache (sparse H2O attention).

Strategy (8 NeuronCores, tensor-parallel over heads, 4 heads/core):

Phase 1 (device, fp32 storage / fp32r matmuls -- 1 cyc/row at free>=256,
same PE speed as bf16 but TF32-grade mantissa so the per-head top-k
selection matches the fp32 reference exactly):
  q/k projections (scale folded into Wq), RoPE, causal-skipped QK^T
  (only k-chunks at/below the diagonal; diagonal chunk gets an additive
  -1e9 triangular mask), exp with row-accumulation, per-head column
  scores  scores[j] = sum_i exp(aw[i,j]) / r_i  via r^T @ E matmuls.

Host: exact top-k per head (jax.lax.top_k tie semantics), gathers kept
x-rows, builds the prefix matrix map.

Phase 2 (device, bf16): the reference output is
    out = sum_kept (aw+1e9) v  -  1e9 * sum_all v      (per row, exactly)
The (aw+1e9) factor is 1e9 * causal-step + O(10) where the O(10) part
is ~1e-9 of the output scale (expected absmax ~1.8e11), far below fp32
resolution of the accumulated sum, so the device computes the step part:
    po[:, i] = 1e9 * sum_{kept j <= i} v_j
which is piecewise-constant in i with <= NKEPT+1 distinct values.  So
phase 2 computes v for kept tokens, prefix sums via a triangular ones
matmul, and only the <=256 distinct o_proj rows per head:
    outS_h = (1e9 * cumsum(v_h)) @ Wo_h          [256, 4096]
Host expands rows (gather), sums the 8 cores' partials, and adds the
exact -1e9 * (sum_all v) @ Wo rank-1 bias row computed in fp64.
"""

import contextlib
import os
import sys

for p in ("/opt/trn_rl_repo", "/root/.axon_site/_ro/trn_rl_repo"):
    if p not in sys.path:
        sys.path.append(p)

import numpy as np
import ml_dtypes

import concourse.bacc as bacc
import concourse.mybir as mybir
import concourse.tile as tile
from concourse.bass_utils import run_bass_kernel_spmd
from concourse.tile import add_dep_helper

F32 = mybir.dt.float32
F32R = mybir.dt.float32r
BF16 = mybir.dt.bfloat16
F16 = mybir.dt.float16
BF = ml_dtypes.bfloat16

P = 128
S = 2048
H = 4096
NH = 32
HD = 128
NCORES = 8
HPC = NH // NCORES          # heads per core = 4
KC = H // P                 # 32 contraction chunks
KEEP = int(0.1 * S)         # 204 top-k heavy hitters
NKEPT = KEEP + 2            # + last-2 local tokens = 206
KPAD = 256                  # padded kept count

_cache = {}


def _r(ap):
    return ap   # operands already declared float32r


def _build_phase1():
    nc = bacc.Bacc("TRN2", target_bir_lowering=False, debug=False,
                   num_devices=NCORES)
    xt = nc.dram_tensor("xt", [H, S], F32R, kind="ExternalInput").ap()
    wqk = nc.dram_tensor("wqk", [H, 2 * HPC * HD], F32R,
                         kind="ExternalInput").ap()
    cosd = nc.dram_tensor("cos", [P, S], F32, kind="ExternalInput").ap()
    sind = nc.dram_tensor("sin", [P, S], F32, kind="ExternalInput").ap()
    dmaskd = nc.dram_tensor("dmask", [P, P], F32,
                            kind="ExternalInput").ap()
    scores_o = nc.dram_tensor("scores", [HPC, S], F32,
                              kind="ExternalOutput").ap()

    with tile.TileContext(nc) as tc, contextlib.ExitStack() as ctx, \
         nc.allow_low_precision(reason="hand-analyzed tf32 score path"):
        # DRAM round-trip buffer for roped q/k (dep-tracked tile):
        # rows [0:512] = qT (4 heads x 128 d), rows [512:1024] = kT.
        dpool = ctx.enter_context(tc.tile_pool(name="dpool", bufs=1,
                                               space="DRAM"))
        qkd = dpool.tile([2 * HPC * HD, S], F32R, name="qkd", tag="qkd")

        # ---- stage A: projections + rope (PE: 32kc x 8 x 4 s-chunks)
        with tc.tile_pool(name="wpool", bufs=1) as wpool, \
             tc.tile_pool(name="cpool", bufs=1) as cpool, \
             tc.tile_pool(name="xpool", bufs=4) as xpool, \
             tc.tile_pool(name="rpool", bufs=2) as rpool, \
             tc.tile_pool(name="stpool", bufs=4) as stpool, \
             tc.tile_pool(name="ppool", bufs=1, space="PSUM") as ppool:
            # weights on the scalar+gpsimd DGE queues so x chunks
            # (sync queue) are not stuck behind 16.8 MB of weight traffic
            w_sb = []
            wq_engines = (nc.scalar, nc.gpsimd)
            for kc in range(KC):
                t = wpool.tile([P, 2 * HPC * HD], F32R, name=f"w{kc}",
                               tag=f"w{kc}")
                wq_engines[kc % 2].dma_start(t[:], wqk[kc * P:(kc + 1) * P, :])
                w_sb.append(t)
            # cos/sin behind the gpsimd weight stream; needed only at ~50us
            cos_sb = cpool.tile([P, S], F32, name="cos_sb", tag="cos")
            sin_sb = cpool.tile([P, S], F32, name="sin_sb", tag="sin")
            nc.gpsimd.dma_start(cos_sb[:], cosd[:, :])
            nc.gpsimd.dma_start(sin_sb[:], sind[:, :])
            IORD = (4, 0, 5, 1, 6, 2, 7, 3)   # k-heads first
            for sq in range(4):
                ssl = slice(sq * 512, (sq + 1) * 512)
                ps = [ppool.tile([P, 512], F32, name=f"pj{i}", tag=f"pj{i}")
                      for i in range(8)]
                for kc in range(KC):
                    xc = xpool.tile([P, 512], F32R, name="xc", tag="xc")
                    nc.sync.dma_start(xc[:], xt[kc * P:(kc + 1) * P, ssl])
                    for i in IORD:
                        nc.tensor.matmul(
                            ps[i][:], lhsT=_r(w_sb[kc][:, i * HD:(i + 1) * HD]),
                            rhs=_r(xc[:]), start=(kc == 0), stop=(kc == KC - 1))
                # first release all 8 PSUM banks with plain copies
                # (alternating DVE/ACT), then do the rope math on the copies
                cps = {}
                for n, i in enumerate(IORD):
                    cp = rpool.tile([P, 512], F32, name="ropecp",
                                    tag=f"ropecp{i}", bufs=2)
                    if n % 2 == 0:
                        nc.vector.tensor_copy(cp[:], ps[i][:])
                    else:
                        nc.scalar.activation(cp[:], ps[i][:],
                                             mybir.ActivationFunctionType.Copy)
                    cps[i] = cp
                for i in IORD:
                    cp = cps[i]
                    m = rpool.tile([P, 512], F32, name="ropem", tag="ropem")
                    nc.vector.tensor_mul(m[:], cp[:], cos_sb[:, ssl])
                    rot = rpool.tile([P, 512], F32, name="roper", tag="roper")
                    # rotate-half on the otherwise-idle scalar engine
                    nc.scalar.activation(rot[0:64, :], cp[64:128, :],
                                         mybir.ActivationFunctionType.Copy,
                                         scale=-1.0)
                    nc.scalar.activation(rot[64:128, :], cp[0:64, :],
                                         mybir.ActivationFunctionType.Copy,
                                         scale=1.0)
                    rs_ = rpool.tile([P, 512], F32, name="ropes", tag="ropes")
                    nc.vector.tensor_mul(rs_[:], rot[:], sin_sb[:, ssl])
                    st = stpool.tile([P, 512], F32R, name="strope", tag="strope")
                    nc.vector.tensor_add(st[:], m[:], rs_[:])
                    nc.sync.dma_start(qkd[i * HD:(i + 1) * HD, ssl], st[:])

        # ---- stage B: causal QK^T + exp + column scores.
        # Two-pass, software-pipelined quarter-head units: pass 1 computes
        # QK chunks + exp (E retained in SBUF) + row sums; pass 2 (one unit
        # behind) does the r^T E score matmuls.  PSUM: aw 2x[128,1024] +
        # one [1,S] score row = 8 banks.
        with tc.tile_pool(name="ktp", bufs=2) as ktp, \
             tc.tile_pool(name="qbp", bufs=2) as qbp, \
             tc.tile_pool(name="epool", bufs=8) as epool, \
             tc.tile_pool(name="vp", bufs=4) as vp, \
             tc.tile_pool(name="rvp", bufs=16) as rvp, \
             tc.tile_pool(name="dmp", bufs=1) as dmp, \
             tc.tile_pool(name="scp", bufs=2) as scp, \
             tc.tile_pool(name="spool", bufs=1, space="PSUM") as spool, \
             tc.tile_pool(name="apool", bufs=3, space="PSUM") as apool:
            dm_sb = dmp.tile([P, P], F32, name="dm", tag="dm")
            nc.sync.dma_start(dm_sb[:], dmaskd[:, :])
            bias5 = dmp.tile([P, 1], F32, name="bias5", tag="bias5")
            nc.vector.memset(bias5[:], -5.0)

            UQT = 4                 # quarter-head pass-1 units
            ESZ = 7424              # sum of exact causal widths in a unit
            state = {}

            def emit_p1(h, q):
                if q == 0:
                    kt = ktp.tile([P, S], F32R, name="kt", tag="kt")
                    qbt = qbp.tile([P, S], F32R, name="qbt", tag="qbt")
                    # column pieces in sq order on the gpsimd queue: piece sq
                    # is ready as soon as stage A's s-chunk sq is written, and
                    # the queue has no write backlog, so early pieces land
                    # immediately (only piece 3 waits on the rope tail)
                    eng = nc.sync if h == 0 else nc.gpsimd
                    for sq in range(4):
                        kssl = slice(sq * 512, (sq + 1) * 512)
                        eng.dma_start(
                            kt[:, kssl],
                            qkd[(HPC + h) * HD:(HPC + h + 1) * HD, kssl])
                        eng.dma_start(
                            qbt[:, kssl],
                            qkd[h * HD:(h + 1) * HD, kssl])
                    state[h] = {"kt": kt, "qbt": qbt, "rinv": {}, "E": {}}
                st_ = state[h]
                E = epool.tile([P, ESZ], F16, name="Eu", tag="Eu")
                st_["E"][q] = E
                eoff = 0
                for qt in range(UQT * q, UQT * q + UQT):
                    W = (qt + 1) * P        # exact causal width
                    rs_list = []
                    for ci, off in enumerate(range(0, W, 1024)):
                        w = min(1024, W - off)
                        aw = apool.tile([P, 1024], F32, name="aw", tag="aw")
                        for sub in range(0, w, 512):
                            sw = min(512, w - sub)
                            nc.tensor.matmul(
                                aw[:, sub:sub + sw],
                                lhsT=_r(st_["qbt"][:, qt * P:(qt + 1) * P]),
                                rhs=_r(st_["kt"][:, off + sub:off + sub + sw]),
                                start=True, stop=True)
                        if off + w == W:   # triangular mask on the last 128
                            nc.vector.tensor_add(aw[:, w - P:w],
                                                 aw[:, w - P:w], dm_sb[:])
                        rs_ = vp.tile([P, 1], F32, name="rsp",
                                      tag=f"rsp{ci}")
                        # exp(aw - 5): keeps E in fp16 range; the e^-5 scale
                        # cancels exactly in scores = (1/rs) * E
                        nc.scalar.activation(E[:, eoff + off:eoff + off + w],
                                             aw[:, :w],
                                             mybir.ActivationFunctionType.Exp,
                                             bias=bias5[:],
                                             accum_out=rs_[:])
                        rs_list.append(rs_)
                    rtot = rs_list[0]
                    for c in range(1, len(rs_list)):
                        nr = vp.tile([P, 1], F32, name="racc", tag=f"racc{c}")
                        nc.vector.tensor_add(nr[:], rtot[:], rs_list[c][:])
                        rtot = nr
                    rinv = rvp.tile([P, 1], F16, name="rinv", tag="rinv")
                    nc.vector.reciprocal(rinv[:], rtot[:])
                    st_["rinv"][qt] = rinv
                    eoff += W

            def emit_p2(h, half):
                # score matmuls for column half [1024*half, 1024*(half+1))
                st_ = state[h]
                base = 1024 * half
                sc = spool.tile([1, 1024], F32, name="scps", tag="scps")
                if half == 0:
                    st_["scsb"] = scp.tile([1, S], F32, name="scsb",
                                           tag="scsb")
                first_qt = 8 * half
                for qt in range(first_qt, 16):
                    W = (qt + 1) * P
                    w = min(1024, W - base)
                    if w <= 0:
                        continue
                    eoff = sum((t + 1) * P
                               for t in range(UQT * (qt // UQT), qt))
                    E = st_["E"][qt // UQT]
                    rinv = st_["rinv"][qt]
                    for sub in range(0, w, 512):
                        sw = min(512, w - sub)
                        nc.tensor.matmul(
                            sc[:, sub:sub + sw], lhsT=rinv[:],
                            rhs=E[:, eoff + base + sub:
                                    eoff + base + sub + sw],
                            start=(qt == (base + sub) // P),
                            stop=(qt == 15))
                nc.vector.tensor_copy(st_["scsb"][:, base:base + 1024],
                                      sc[:])
                if half == 1:
                    nc.gpsimd.dma_start(scores_o[h:h + 1, :], st_["scsb"][:])

            for h in range(HPC):
                for q in range(4):
                    emit_p1(h, q)
                    if h > 0 and q == 1:
                        emit_p2(h - 1, 0)
                    if h > 0 and q == 3:
                        emit_p2(h - 1, 1)
            emit_p2(HPC - 1, 0)
            emit_p2(HPC - 1, 1)
    nc.compile()
    return nc


def _build_phase2():
    nc = bacc.Bacc("TRN2", target_bir_lowering=False, debug=False,
                   num_devices=NCORES)
    # packed: row p, col kc*W+c holds original row kc*128+p, col c
    xtk = nc.dram_tensor("xtk", [P, KC * HPC * KPAD], BF16,
                         kind="ExternalInput").ap()
    wv = nc.dram_tensor("wv", [P, KC * HPC * HD], BF16,
                        kind="ExternalInput").ap()
    wo = nc.dram_tensor("wo", [HPC * HD, H], BF16, kind="ExternalInput").ap()
    Rmd = nc.dram_tensor("Rm", [KPAD, KPAD], BF16, kind="ExternalInput").ap()
    outS = nc.dram_tensor("outS", [HPC * KPAD, H], BF16,
                          kind="ExternalOutput").ap()

    with tile.TileContext(nc) as tc, contextlib.ExitStack() as ctx:
        wvp = ctx.enter_context(tc.tile_pool(name="wvp", bufs=1))
        wop = ctx.enter_context(tc.tile_pool(name="wop", bufs=1))
        xkp = ctx.enter_context(tc.tile_pool(name="xkp", bufs=1))
        rp = ctx.enter_context(tc.tile_pool(name="rp", bufs=1))
        vsb = ctx.enter_context(tc.tile_pool(name="vsb", bufs=1))
        csb = ctx.enter_context(tc.tile_pool(name="csb", bufs=1))
        osb = ctx.enter_context(tc.tile_pool(name="osb", bufs=6))

        # wv packed into one resident tile (2 big DMAs on the scalar queue)
        WVW = HPC * HD
        wv_big = wvp.tile([P, KC * WVW], BF16, name="wv_big", tag="wv_big")
        NWQ = 4
        for hh in range(NWQ):
            csl = slice(hh * KC * WVW // NWQ, (hh + 1) * KC * WVW // NWQ)
            nc.scalar.dma_start(wv_big[:, csl], wv[:, csl])
        wv_sb = [wv_big[:, kc * WVW:(kc + 1) * WVW] for kc in range(KC)]
        R_sb = [rp.tile([P, KPAD], BF16, name=f"R{t}", tag=f"R{t}")
                for t in range(2)]
        for t in range(2):
            nc.scalar.dma_start(R_sb[t][:], Rmd[t * P:(t + 1) * P, :])

        # v projection of kept tokens: v_sb[h][t] = [128 kept, 128 d] bf16
        v_sb = [[vsb.tile([P, HD], BF16, name=f"vsb{h}_{t}", tag=f"vsb{h}_{t}")
                 for t in range(2)] for h in range(HPC)]
        with tc.tile_pool(name="vps", bufs=1, space="PSUM") as vps:
            v_ps = [[vps.tile([P, HD], F32, name=f"vps{h}_{t}",
                              tag=f"vps{h}_{t}")
                     for t in range(2)] for h in range(HPC)]
            XKW = HPC * KPAD
            xk_big = xkp.tile([P, KC * XKW], BF16, name="xk_big",
                              tag="xk_big")
            xk_insts = []
            NXQ = 32
            for qq in range(NXQ):
                csl = slice(qq * KC * XKW // NXQ,
                            (qq + 1) * KC * XKW // NXQ)
                xk_insts.append(nc.sync.dma_start(xk_big[:, csl],
                                                  xtk[:, csl]))
            for kc in range(KC):
                xk = xk_big[:, kc * XKW:(kc + 1) * XKW]
                for h in range(HPC):
                    for t in range(2):
                        nc.tensor.matmul(
                            v_ps[h][t][:],
                            lhsT=xk[:, h * KPAD + t * P:h * KPAD + (t + 1) * P],
                            rhs=wv_sb[kc][:, h * HD:(h + 1) * HD],
                            start=(kc == 0), stop=(kc == KC - 1))
            for h in range(HPC):
                for t in range(2):
                    nc.vector.tensor_copy(v_sb[h][t][:], v_ps[h][t][:])

        # wo loads emitted after the v-proj stream so they don't delay it
        wo_sb = [wop.tile([P, H], BF16, name=f"wo{h}", tag=f"wo{h}")
                 for h in range(HPC)]
        for h in range(HPC):
            wo_i = nc.gpsimd.dma_start(wo_sb[h][:], wo[h * P:(h + 1) * P, :])
            # hold the 4 MB wo stream until the v-proj input stream is
            # mostly through the DMA engines
            add_dep_helper(xk_insts[len(xk_insts) * 3 // 4].ins, wo_i.ins,
                           reason="wo transfers after v-proj inputs")

        # prefix sums over sorted kept order: cumT[h] = [128 d, 256 m] bf16,
        # scaled by 1e9 on the PSUM->SBUF copy.
        cum_sb = [csb.tile([P, KPAD], BF16, name=f"cum{h}", tag=f"cum{h}")
                  for h in range(HPC)]
        with tc.tile_pool(name="cps", bufs=1, space="PSUM") as cps:
            for h in range(HPC):
                cum_ps = cps.tile([P, KPAD], F32, name="cumps", tag=f"cps{h}")
                for t in range(2):
                    nc.tensor.matmul(cum_ps[:], lhsT=v_sb[h][t][:],
                                     rhs=R_sb[t][:],
                                     start=(t == 0), stop=(t == 1))
                nc.scalar.activation(cum_sb[h][:], cum_ps[:],
                                     mybir.ActivationFunctionType.Copy,
                                     scale=1e9)

        # distinct o_proj rows: outS[h*256+m, :] = cumT[h][:, m] @ wo_h
        # only m <= NKEPT=206 is ever gathered, so the mb=1 block writes
        # just its first 80 rows.
        with tc.tile_pool(name="ops", bufs=4, space="PSUM") as ops:
            for h in range(HPC):
                for mb in range(2):
                    rows = P if mb == 0 else (NKEPT - P + 2)
                    ob = osb.tile([P, H], BF16, name="ob", tag="ob")
                    for nt in range(8):
                        nsl = slice(nt * 512, (nt + 1) * 512)
                        o_ps = ops.tile([P, 512], F32, name="ops_t", tag="ops_t")
                        nc.tensor.matmul(
                            o_ps[:], lhsT=cum_sb[h][:, mb * P:(mb + 1) * P],
                            rhs=wo_sb[h][:, nsl], start=True, stop=True)
                        if nt % 2 == 0:
                            nc.vector.tensor_copy(ob[:rows, nsl],
                                                  o_ps[:rows, :])
                        else:
                            nc.scalar.activation(
                                ob[:rows, nsl], o_ps[:rows, :],
                                mybir.ActivationFunctionType.Copy)
                    nc.sync.dma_start(
                        outS[(h * 2 + mb) * P:(h * 2 + mb) * P + rows, :],
                        ob[:rows, :])
    nc.compile()
    return nc


def _topk_kept(scores_h):
    """jax.lax.top_k semantics: descending, ties -> lower index."""
    s = scores_h[:-2]
    idx = np.argsort(-s, kind="stable")[:KEEP]
    kept = np.concatenate([idx, [S - 2, S - 1]])
    kept.sort()
    return kept.astype(np.int64)


def kernel(hidden_states, attention_mask, Wq, Wk, Wv, Wo, position_ids):
    x = np.ascontiguousarray(np.asarray(hidden_states, np.float32)[0])  # [S,H]
    Wq = np.asarray(Wq, np.float32)
    Wk = np.asarray(Wk, np.float32)
    Wv = np.asarray(Wv, np.float32)
    Wo = np.asarray(Wo, np.float32)
    pos = np.asarray(position_ids)[0]

    inv = 1.0 / (10000.0 ** (np.arange(0, HD, 2, dtype=np.float32) / HD))
    fr = pos.astype(np.float32)[:, None] * inv
    emb = np.concatenate([fr, fr], -1)
    cosT = np.ascontiguousarray(np.cos(emb).astype(np.float32).T)  # [128, S]
    sinT = np.ascontiguousarray(np.sin(emb).astype(np.float32).T)
    xT = np.ascontiguousarray(x.T)                                 # [H, S]
    scale = np.float32(1.0 / np.sqrt(HD))

    # diagonal-chunk masks: for qt%4 == j, cols (of the 512-wide chunk)
    # beyond j*128+row are masked with -1e9
    row = np.arange(P)[:, None]
    col = np.arange(P)[None, :]
    dmask = np.where(col <= row, 0.0, np.float32(-1e9)).astype(np.float32)

    if "p1" not in _cache:
        _cache["p1"] = _build_phase1()
    nc1 = _cache["p1"]

    in_maps = []
    for c in range(NCORES):
        hsl = slice(c * HPC * HD, (c + 1) * HPC * HD)
        wqk = np.concatenate([Wq[hsl, :].T * scale, Wk[hsl, :].T],
                             axis=1).astype(np.float32)
        in_maps.append({
            "xt": xT, "wqk": np.ascontiguousarray(wqk),
            "cos": cosT, "sin": sinT, "dmask": dmask,
        })
    _tr = bool(int(os.environ.get("KTRACE", "0")))
    r1 = run_bass_kernel_spmd(nc1, in_maps, list(range(NCORES)), trace=_tr)
    _cache["exec1"] = r1.exec_time_ns

    # host: top-k, gathers, prefix maps
    x_bf = x.astype(BF)
    Rm = np.triu(np.ones((KPAD, KPAD), np.float32), 1).astype(BF)
    in_maps2, midx = [], []
    for c in range(NCORES):
        scores = r1.results[c]["scores"]
        hsl = slice(c * HPC * HD, (c + 1) * HPC * HD)
        xtkv = np.zeros((H, HPC * KPAD), BF)
        mrows = []
        for h in range(HPC):
            kept = _topk_kept(scores[h])
            xtkv[:, h * KPAD:h * KPAD + NKEPT] = x_bf[kept, :].T
            mrows.append(np.searchsorted(kept, np.arange(S), side="right"))
        midx.append(mrows)
        xtkp = np.ascontiguousarray(
            xtkv.reshape(KC, P, HPC * KPAD).transpose(1, 0, 2)
                .reshape(P, KC * HPC * KPAD))
        wvp_ = np.ascontiguousarray(
            Wv[hsl, :].T.astype(BF).reshape(KC, P, HPC * HD)
            .transpose(1, 0, 2).reshape(P, KC * HPC * HD))
        in_maps2.append({
            "xtk": xtkp,
            "wv": wvp_,
            "wo": np.ascontiguousarray(Wo[:, hsl].T).astype(BF),
            "Rm": Rm,
        })

    if "p2" not in _cache:
        _cache["p2"] = _build_phase2()
    nc2 = _cache["p2"]
    r2 = run_bass_kernel_spmd(nc2, in_maps2, list(range(NCORES)), trace=_tr)
    _cache["exec2"] = r2.exec_time_ns

    # host: expand piecewise-constant rows, sum cores, add exact bias row
    acc = np.zeros((S, H), np.float32)
    for c in range(NCORES):
        oS = np.asarray(r2.results[c]["outS"]).astype(np.float32)
        for h in range(HPC):
            acc += oS[h * KPAD:(h + 1) * KPAD][midx[c][h]]
    xsum = x.astype(np.float64).sum(0)
    vsum = xsum @ Wv.astype(np.float64).T
    bias = (-1e9 * (vsum @ Wo.astype(np.float64).T)).astype(np.float32)
    acc += bias[None, :]
    return acc.reshape(1, S, H)


# revision 52
# speedup vs baseline: 1.1733x; 1.0096x over previous
"""Trainium2 Bass kernel for nn_LlamaAttention_kvcache (sparse H2O attention).

Strategy (8 NeuronCores, tensor-parallel over heads, 4 heads/core):

Phase 1 (device, fp32 storage / fp32r matmuls -- 1 cyc/row at free>=256,
same PE speed as bf16 but TF32-grade mantissa so the per-head top-k
selection matches the fp32 reference exactly):
  q/k projections (scale folded into Wq), RoPE, causal-skipped QK^T
  (only k-chunks at/below the diagonal; diagonal chunk gets an additive
  -1e9 triangular mask), exp with row-accumulation, per-head column
  scores  scores[j] = sum_i exp(aw[i,j]) / r_i  via r^T @ E matmuls.

Host: exact top-k per head (jax.lax.top_k tie semantics), gathers kept
x-rows, builds the prefix matrix map.

Phase 2 (device, bf16): the reference output is
    out = sum_kept (aw+1e9) v  -  1e9 * sum_all v      (per row, exactly)
The (aw+1e9) factor is 1e9 * causal-step + O(10) where the O(10) part
is ~1e-9 of the output scale (expected absmax ~1.8e11), far below fp32
resolution of the accumulated sum, so the device computes the step part:
    po[:, i] = 1e9 * sum_{kept j <= i} v_j
which is piecewise-constant in i with <= NKEPT+1 distinct values.  So
phase 2 computes v for kept tokens, prefix sums via a triangular ones
matmul, and only the <=256 distinct o_proj rows per head:
    outS_h = (1e9 * cumsum(v_h)) @ Wo_h          [256, 4096]
Host expands rows (gather), sums the 8 cores' partials, and adds the
exact -1e9 * (sum_all v) @ Wo rank-1 bias row computed in fp64.
"""

import contextlib
import os
import sys

for p in ("/opt/trn_rl_repo", "/root/.axon_site/_ro/trn_rl_repo"):
    if p not in sys.path:
        sys.path.append(p)

import numpy as np
import ml_dtypes

import concourse.bacc as bacc
import concourse.mybir as mybir
import concourse.tile as tile
from concourse.bass_utils import run_bass_kernel_spmd
from concourse.tile import add_dep_helper

F32 = mybir.dt.float32
F32R = mybir.dt.float32r
BF16 = mybir.dt.bfloat16
F16 = mybir.dt.float16
BF = ml_dtypes.bfloat16

P = 128
S = 2048
H = 4096
NH = 32
HD = 128
NCORES = 8
HPC = NH // NCORES          # heads per core = 4
KC = H // P                 # 32 contraction chunks
KEEP = int(0.1 * S)         # 204 top-k heavy hitters
NKEPT = KEEP + 2            # + last-2 local tokens = 206
KPAD = 208                  # padded kept count (128 + 80)

_cache = {}


def _r(ap):
    return ap   # operands already declared float32r


def _build_phase1():
    nc = bacc.Bacc("TRN2", target_bir_lowering=False, debug=False,
                   num_devices=NCORES)
    xt = nc.dram_tensor("xt", [H, S], F32R, kind="ExternalInput").ap()
    wqk = nc.dram_tensor("wqk", [H, 2 * HPC * HD], F32R,
                         kind="ExternalInput").ap()
    cosd = nc.dram_tensor("cos", [P, S], F32, kind="ExternalInput").ap()
    sind = nc.dram_tensor("sin", [P, S], F32, kind="ExternalInput").ap()
    dmaskd = nc.dram_tensor("dmask", [P, P], F32,
                            kind="ExternalInput").ap()
    scores_o = nc.dram_tensor("scores", [HPC, S], F32,
                              kind="ExternalOutput").ap()

    with tile.TileContext(nc) as tc, contextlib.ExitStack() as ctx, \
         nc.allow_low_precision(reason="hand-analyzed tf32 score path"):
        # DRAM round-trip buffer for roped q/k (dep-tracked tile):
        # rows [0:512] = qT (4 heads x 128 d), rows [512:1024] = kT.
        dpool = ctx.enter_context(tc.tile_pool(name="dpool", bufs=1,
                                               space="DRAM"))
        qkd = dpool.tile([2 * HPC * HD, S], F32R, name="qkd", tag="qkd")

        # ---- stage A: projections + rope (PE: 32kc x 8 x 4 s-chunks)
        with tc.tile_pool(name="wpool", bufs=1) as wpool, \
             tc.tile_pool(name="cpool", bufs=1) as cpool, \
             tc.tile_pool(name="xpool", bufs=4) as xpool, \
             tc.tile_pool(name="rpool", bufs=2) as rpool, \
             tc.tile_pool(name="stpool", bufs=4) as stpool, \
             tc.tile_pool(name="ppool", bufs=1, space="PSUM") as ppool:
            # weights on the scalar+gpsimd DGE queues so x chunks
            # (sync queue) are not stuck behind 16.8 MB of weight traffic
            w_sb = []
            wq_engines = (nc.scalar, nc.gpsimd)
            for kc in range(KC):
                t = wpool.tile([P, 2 * HPC * HD], F32R, name=f"w{kc}",
                               tag=f"w{kc}")
                wq_engines[kc % 2].dma_start(t[:], wqk[kc * P:(kc + 1) * P, :])
                w_sb.append(t)
            # cos/sin behind the gpsimd weight stream; needed only at ~50us
            cos_sb = cpool.tile([P, S], F32, name="cos_sb", tag="cos")
            sin_sb = cpool.tile([P, S], F32, name="sin_sb", tag="sin")
            nc.gpsimd.dma_start(cos_sb[:], cosd[:, :])
            nc.gpsimd.dma_start(sin_sb[:], sind[:, :])
            IORD = (4, 0, 5, 1, 6, 2, 7, 3)   # k-heads first
            for sq in range(4):
                ssl = slice(sq * 512, (sq + 1) * 512)
                ps = [ppool.tile([P, 512], F32, name=f"pj{i}", tag=f"pj{i}")
                      for i in range(8)]
                for kc in range(KC):
                    xc = xpool.tile([P, 512], F32R, name="xc", tag="xc")
                    nc.sync.dma_start(xc[:], xt[kc * P:(kc + 1) * P, ssl])
                    for i in IORD:
                        nc.tensor.matmul(
                            ps[i][:], lhsT=_r(w_sb[kc][:, i * HD:(i + 1) * HD]),
                            rhs=_r(xc[:]), start=(kc == 0), stop=(kc == KC - 1))
                # first release all 8 PSUM banks with plain copies
                # (alternating DVE/ACT), then do the rope math on the copies
                cps = {}
                for n, i in enumerate(IORD):
                    cp = rpool.tile([P, 512], F32, name="ropecp",
                                    tag=f"ropecp{i}", bufs=2)
                    if n % 2 == 0:
                        nc.vector.tensor_copy(cp[:], ps[i][:])
                    else:
                        nc.scalar.activation(cp[:], ps[i][:],
                                             mybir.ActivationFunctionType.Copy)
                    cps[i] = cp
                for i in IORD:
                    cp = cps[i]
                    m = rpool.tile([P, 512], F32, name="ropem", tag="ropem")
                    nc.vector.tensor_mul(m[:], cp[:], cos_sb[:, ssl])
                    rot = rpool.tile([P, 512], F32, name="roper", tag="roper")
                    # rotate-half on the otherwise-idle scalar engine
                    nc.scalar.activation(rot[0:64, :], cp[64:128, :],
                                         mybir.ActivationFunctionType.Copy,
                                         scale=-1.0)
                    nc.scalar.activation(rot[64:128, :], cp[0:64, :],
                                         mybir.ActivationFunctionType.Copy,
                                         scale=1.0)
                    rs_ = rpool.tile([P, 512], F32, name="ropes", tag="ropes")
                    nc.vector.tensor_mul(rs_[:], rot[:], sin_sb[:, ssl])
                    st = stpool.tile([P, 512], F32R, name="strope", tag="strope")
                    nc.vector.tensor_add(st[:], m[:], rs_[:])
                    nc.sync.dma_start(qkd[i * HD:(i + 1) * HD, ssl], st[:])

        # ---- stage B: causal QK^T + exp + column scores.
        # Two-pass, software-pipelined quarter-head units: pass 1 computes
        # QK chunks + exp (E retained in SBUF) + row sums; pass 2 (one unit
        # behind) does the r^T E score matmuls.  PSUM: aw 2x[128,1024] +
        # one [1,S] score row = 8 banks.
        with tc.tile_pool(name="ktp", bufs=2) as ktp, \
             tc.tile_pool(name="qbp", bufs=2) as qbp, \
             tc.tile_pool(name="epool", bufs=8) as epool, \
             tc.tile_pool(name="vp", bufs=4) as vp, \
             tc.tile_pool(name="rvp", bufs=16) as rvp, \
             tc.tile_pool(name="dmp", bufs=1) as dmp, \
             tc.tile_pool(name="scp", bufs=2) as scp, \
             tc.tile_pool(name="spool", bufs=1, space="PSUM") as spool, \
             tc.tile_pool(name="apool", bufs=3, space="PSUM") as apool:
            dm_sb = dmp.tile([P, P], F32, name="dm", tag="dm")
            nc.sync.dma_start(dm_sb[:], dmaskd[:, :])
            bias5 = dmp.tile([P, 1], F32, name="bias5", tag="bias5")
            nc.vector.memset(bias5[:], -5.0)

            UQT = 4                 # quarter-head pass-1 units
            ESZ = 7424              # sum of exact causal widths in a unit
            state = {}

            def emit_p1(h, q):
                if q == 0:
                    kt = ktp.tile([P, S], F32R, name="kt", tag="kt")
                    qbt = qbp.tile([P, S], F32R, name="qbt", tag="qbt")
                    # column pieces in sq order on the gpsimd queue: piece sq
                    # is ready as soon as stage A's s-chunk sq is written, and
                    # the queue has no write backlog, so early pieces land
                    # immediately (only piece 3 waits on the rope tail)
                    eng = nc.sync if h == 0 else nc.gpsimd
                    for sq in range(4):
                        kssl = slice(sq * 512, (sq + 1) * 512)
                        eng.dma_start(
                            kt[:, kssl],
                            qkd[(HPC + h) * HD:(HPC + h + 1) * HD, kssl])
                        eng.dma_start(
                            qbt[:, kssl],
                            qkd[h * HD:(h + 1) * HD, kssl])
                    state[h] = {"kt": kt, "qbt": qbt, "rinv": {}, "E": {}}
                st_ = state[h]
                E = epool.tile([P, ESZ], F16, name="Eu", tag="Eu")
                st_["E"][q] = E
                eoff = 0
                for qt in range(UQT * q, UQT * q + UQT):
                    W = (qt + 1) * P        # exact causal width
                    rs_list = []
                    for ci, off in enumerate(range(0, W, 1024)):
                        w = min(1024, W - off)
                        aw = apool.tile([P, 1024], F32, name="aw", tag="aw")
                        for sub in range(0, w, 512):
                            sw = min(512, w - sub)
                            nc.tensor.matmul(
                                aw[:, sub:sub + sw],
                                lhsT=_r(st_["qbt"][:, qt * P:(qt + 1) * P]),
                                rhs=_r(st_["kt"][:, off + sub:off + sub + sw]),
                                start=True, stop=True)
                        if off + w == W:   # triangular mask on the last 128
                            nc.vector.tensor_add(aw[:, w - P:w],
                                                 aw[:, w - P:w], dm_sb[:])
                        rs_ = vp.tile([P, 1], F32, name="rsp",
                                      tag=f"rsp{ci}")
                        # exp(aw - 5): keeps E in fp16 range; the e^-5 scale
                        # cancels exactly in scores = (1/rs) * E
                        nc.scalar.activation(E[:, eoff + off:eoff + off + w],
                                             aw[:, :w],
                                             mybir.ActivationFunctionType.Exp,
                                             bias=bias5[:],
                                             accum_out=rs_[:])
                        rs_list.append(rs_)
                    rtot = rs_list[0]
                    for c in range(1, len(rs_list)):
                        nr = vp.tile([P, 1], F32, name="racc", tag=f"racc{c}")
                        nc.vector.tensor_add(nr[:], rtot[:], rs_list[c][:])
                        rtot = nr
                    rinv = rvp.tile([P, 1], F16, name="rinv", tag="rinv")
                    nc.vector.reciprocal(rinv[:], rtot[:])
                    st_["rinv"][qt] = rinv
                    eoff += W

            def emit_p2(h, half):
                # score matmuls for column half [1024*half, 1024*(half+1))
                st_ = state[h]
                base = 1024 * half
                sc = spool.tile([1, 1024], F32, name="scps", tag="scps")
                if half == 0:
                    st_["scsb"] = scp.tile([1, S], F32, name="scsb",
                                           tag="scsb")
                first_qt = 8 * half
                for qt in range(first_qt, 16):
                    W = (qt + 1) * P
                    w = min(1024, W - base)
                    if w <= 0:
                        continue
                    eoff = sum((t + 1) * P
                               for t in range(UQT * (qt // UQT), qt))
                    E = st_["E"][qt // UQT]
                    rinv = st_["rinv"][qt]
                    for sub in range(0, w, 512):
                        sw = min(512, w - sub)
                        nc.tensor.matmul(
                            sc[:, sub:sub + sw], lhsT=rinv[:],
                            rhs=E[:, eoff + base + sub:
                                    eoff + base + sub + sw],
                            start=(qt == (base + sub) // P),
                            stop=(qt == 15))
                nc.vector.tensor_copy(st_["scsb"][:, base:base + 1024],
                                      sc[:])
                if half == 1:
                    nc.gpsimd.dma_start(scores_o[h:h + 1, :], st_["scsb"][:])

            for h in range(HPC):
                for q in range(4):
                    emit_p1(h, q)
                    if h > 0 and q == 1:
                        emit_p2(h - 1, 0)
                    if h > 0 and q == 3:
                        emit_p2(h - 1, 1)
            emit_p2(HPC - 1, 0)
            emit_p2(HPC - 1, 1)
    nc.compile()
    return nc


def _build_phase2():
    nc = bacc.Bacc("TRN2", target_bir_lowering=False, debug=False,
                   num_devices=NCORES)
    # packed: row p, col kc*W+c holds original row kc*128+p, col c
    xtk = nc.dram_tensor("xtk", [P, KC * HPC * KPAD], BF16,
                         kind="ExternalInput").ap()
    wv = nc.dram_tensor("wv", [P, KC * HPC * HD], BF16,
                        kind="ExternalInput").ap()
    wo = nc.dram_tensor("wo", [HPC * HD, H], BF16, kind="ExternalInput").ap()
    Rmd = nc.dram_tensor("Rm", [KPAD, KPAD], BF16, kind="ExternalInput").ap()
    outS = nc.dram_tensor("outS", [HPC * KPAD, H], BF16,
                          kind="ExternalOutput").ap()

    with tile.TileContext(nc) as tc, contextlib.ExitStack() as ctx:
        wvp = ctx.enter_context(tc.tile_pool(name="wvp", bufs=1))
        wop = ctx.enter_context(tc.tile_pool(name="wop", bufs=1))
        xkp = ctx.enter_context(tc.tile_pool(name="xkp", bufs=1))
        rp = ctx.enter_context(tc.tile_pool(name="rp", bufs=1))
        vsb = ctx.enter_context(tc.tile_pool(name="vsb", bufs=1))
        csb = ctx.enter_context(tc.tile_pool(name="csb", bufs=1))
        osb = ctx.enter_context(tc.tile_pool(name="osb", bufs=6))

        # wv packed into one resident tile (2 big DMAs on the scalar queue)
        WVW = HPC * HD
        wv_big = wvp.tile([P, KC * WVW], BF16, name="wv_big", tag="wv_big")
        NWQ = 4
        for hh in range(NWQ):
            csl = slice(hh * KC * WVW // NWQ, (hh + 1) * KC * WVW // NWQ)
            nc.scalar.dma_start(wv_big[:, csl], wv[:, csl])
        wv_sb = [wv_big[:, kc * WVW:(kc + 1) * WVW] for kc in range(KC)]
        R_sb = [rp.tile([(P, KPAD - P)[t], KPAD], BF16, name=f"R{t}",
                        tag=f"R{t}")
                for t in range(2)]
        for t in range(2):
            nc.scalar.dma_start(R_sb[t][:],
                                Rmd[t * P:t * P + (P, KPAD - P)[t], :])

        # v projection of kept tokens: v_sb[h][t] = [128 kept, 128 d] bf16
        TROWS = (P, KPAD - P)   # 128 + 80 kept rows
        v_sb = [[vsb.tile([TROWS[t], HD], BF16, name=f"vsb{h}_{t}",
                          tag=f"vsb{h}_{t}")
                 for t in range(2)] for h in range(HPC)]
        with tc.tile_pool(name="vps", bufs=1, space="PSUM") as vps:
            v_ps = [[vps.tile([TROWS[t], HD], F32, name=f"vps{h}_{t}",
                              tag=f"vps{h}_{t}")
                     for t in range(2)] for h in range(HPC)]
            XKW = HPC * KPAD
            xk_big = xkp.tile([P, KC * XKW], BF16, name="xk_big",
                              tag="xk_big")
            xk_insts = []
            NXQ = 32
            for qq in range(NXQ):
                csl = slice(qq * KC * XKW // NXQ,
                            (qq + 1) * KC * XKW // NXQ)
                xk_insts.append(nc.sync.dma_start(xk_big[:, csl],
                                                  xtk[:, csl]))
            for kc in range(KC):
                xk = xk_big[:, kc * XKW:(kc + 1) * XKW]
                for h in range(HPC):
                    for t in range(2):
                        nc.tensor.matmul(
                            v_ps[h][t][:],
                            lhsT=xk[:, h * KPAD + t * P:
                                    h * KPAD + t * P + TROWS[t]],
                            rhs=wv_sb[kc][:, h * HD:(h + 1) * HD],
                            start=(kc == 0), stop=(kc == KC - 1))
            for h in range(HPC):
                for t in range(2):
                    nc.vector.tensor_copy(v_sb[h][t][:], v_ps[h][t][:])

        # wo loads emitted after the v-proj stream so they don't delay it
        wo_sb = [wop.tile([P, H], BF16, name=f"wo{h}", tag=f"wo{h}")
                 for h in range(HPC)]
        for h in range(HPC):
            wo_i = nc.gpsimd.dma_start(wo_sb[h][:], wo[h * P:(h + 1) * P, :])
            # hold the 4 MB wo stream until the v-proj input stream is
            # mostly through the DMA engines
            add_dep_helper(xk_insts[len(xk_insts) * 3 // 4].ins, wo_i.ins,
                           reason="wo transfers after v-proj inputs")

        # prefix sums over sorted kept order: cumT[h] = [128 d, 256 m] bf16,
        # scaled by 1e9 on the PSUM->SBUF copy.
        cum_sb = [csb.tile([P, KPAD], BF16, name=f"cum{h}", tag=f"cum{h}")
                  for h in range(HPC)]
        with tc.tile_pool(name="cps", bufs=1, space="PSUM") as cps:
            for h in range(HPC):
                cum_ps = cps.tile([P, KPAD], F32, name="cumps", tag=f"cps{h}")
                for t in range(2):
                    nc.tensor.matmul(cum_ps[:], lhsT=v_sb[h][t][:],
                                     rhs=R_sb[t][:],
                                     start=(t == 0), stop=(t == 1))
                nc.scalar.activation(cum_sb[h][:], cum_ps[:],
                                     mybir.ActivationFunctionType.Copy,
                                     scale=1e9)

        # distinct o_proj rows: outS[h*256+m, :] = cumT[h][:, m] @ wo_h
        # only m <= NKEPT=206 is ever gathered, so the mb=1 block writes
        # just its first 80 rows.
        with tc.tile_pool(name="ops", bufs=4, space="PSUM") as ops:
            for h in range(HPC):
                for mb in range(2):
                    rows = P if mb == 0 else (NKEPT - P + 2)
                    ob = osb.tile([P, H], BF16, name="ob", tag="ob")
                    for nt in range(8):
                        nsl = slice(nt * 512, (nt + 1) * 512)
                        o_ps = ops.tile([P, 512], F32, name="ops_t", tag="ops_t")
                        nc.tensor.matmul(
                            o_ps[:rows, :],
                            lhsT=cum_sb[h][:, mb * P:mb * P + rows],
                            rhs=wo_sb[h][:, nsl], start=True, stop=True)
                        if nt % 2 == 0:
                            nc.vector.tensor_copy(ob[:rows, nsl],
                                                  o_ps[:rows, :])
                        else:
                            nc.scalar.activation(
                                ob[:rows, nsl], o_ps[:rows, :],
                                mybir.ActivationFunctionType.Copy)
                    nc.sync.dma_start(
                        outS[h * KPAD + mb * P:h * KPAD + mb * P + rows, :],
                        ob[:rows, :])
    nc.compile()
    return nc


def _topk_kept(scores_h):
    """jax.lax.top_k semantics: descending, ties -> lower index."""
    s = scores_h[:-2]
    idx = np.argsort(-s, kind="stable")[:KEEP]
    kept = np.concatenate([idx, [S - 2, S - 1]])
    kept.sort()
    return kept.astype(np.int64)


def kernel(hidden_states, attention_mask, Wq, Wk, Wv, Wo, position_ids):
    x = np.ascontiguousarray(np.asarray(hidden_states, np.float32)[0])  # [S,H]
    Wq = np.asarray(Wq, np.float32)
    Wk = np.asarray(Wk, np.float32)
    Wv = np.asarray(Wv, np.float32)
    Wo = np.asarray(Wo, np.float32)
    pos = np.asarray(position_ids)[0]

    inv = 1.0 / (10000.0 ** (np.arange(0, HD, 2, dtype=np.float32) / HD))
    fr = pos.astype(np.float32)[:, None] * inv
    emb = np.concatenate([fr, fr], -1)
    cosT = np.ascontiguousarray(np.cos(emb).astype(np.float32).T)  # [128, S]
    sinT = np.ascontiguousarray(np.sin(emb).astype(np.float32).T)
    xT = np.ascontiguousarray(x.T)                                 # [H, S]
    scale = np.float32(1.0 / np.sqrt(HD))

    # diagonal-chunk masks: for qt%4 == j, cols (of the 512-wide chunk)
    # beyond j*128+row are masked with -1e9
    row = np.arange(P)[:, None]
    col = np.arange(P)[None, :]
    dmask = np.where(col <= row, 0.0, np.float32(-1e9)).astype(np.float32)

    if "p1" not in _cache:
        _cache["p1"] = _build_phase1()
    nc1 = _cache["p1"]

    in_maps = []
    for c in range(NCORES):
        hsl = slice(c * HPC * HD, (c + 1) * HPC * HD)
        wqk = np.concatenate([Wq[hsl, :].T * scale, Wk[hsl, :].T],
                             axis=1).astype(np.float32)
        in_maps.append({
            "xt": xT, "wqk": np.ascontiguousarray(wqk),
            "cos": cosT, "sin": sinT, "dmask": dmask,
        })
    _tr = bool(int(os.environ.get("KTRACE", "0")))
    r1 = run_bass_kernel_spmd(nc1, in_maps, list(range(NCORES)), trace=_tr)
    _cache["exec1"] = r1.exec_time_ns

    # host: top-k, gathers, prefix maps
    x_bf = x.astype(BF)
    Rm = np.triu(np.ones((KPAD, KPAD), np.float32), 1).astype(BF)
    in_maps2, midx = [], []
    for c in range(NCORES):
        scores = r1.results[c]["scores"]
        hsl = slice(c * HPC * HD, (c + 1) * HPC * HD)
        xtkv = np.zeros((H, HPC * KPAD), BF)
        mrows = []
        for h in range(HPC):
            kept = _topk_kept(scores[h])
            xtkv[:, h * KPAD:h * KPAD + NKEPT] = x_bf[kept, :].T
            mrows.append(np.searchsorted(kept, np.arange(S), side="right"))
        midx.append(mrows)
        xtkp = np.ascontiguousarray(
            xtkv.reshape(KC, P, HPC * KPAD).transpose(1, 0, 2)
                .reshape(P, KC * HPC * KPAD))
        wvp_ = np.ascontiguousarray(
            Wv[hsl, :].T.astype(BF).reshape(KC, P, HPC * HD)
            .transpose(1, 0, 2).reshape(P, KC * HPC * HD))
        in_maps2.append({
            "xtk": xtkp,
            "wv": wvp_,
            "wo": np.ascontiguousarray(Wo[:, hsl].T).astype(BF),
            "Rm": Rm,
        })

    if "p2" not in _cache:
        _cache["p2"] = _build_phase2()
    nc2 = _cache["p2"]
    r2 = run_bass_kernel_spmd(nc2, in_maps2, list(range(NCORES)), trace=_tr)
    _cache["exec2"] = r2.exec_time_ns

    # host: expand piecewise-constant rows, sum cores, add exact bias row
    acc = np.zeros((S, H), np.float32)
    for c in range(NCORES):
        oS = np.asarray(r2.results[c]["outS"]).astype(np.float32)
        for h in range(HPC):
            acc += oS[h * KPAD:(h + 1) * KPAD][midx[c][h]]
    xsum = x.astype(np.float64).sum(0)
    vsum = xsum @ Wv.astype(np.float64).T
    bias = (-1e9 * (vsum @ Wo.astype(np.float64).T)).astype(np.float32)
    acc += bias[None, :]
    return acc.reshape(1, S, H)
